# revision 10
# baseline (speedup 1.0000x reference)
"""GCN 2-layer kernel for trn2: host preprocessing + Bass kernel builder.

Math (per GCNConv, PyG-style):
  out = D^-1/2 (A+I) D^-1/2 (X W) + b
Layer1 -> relu -> Layer2.

Device plan (8 cores, SPMD), all tensors in pos-order with p-major DRAM
layout [128, NB, feat] (row of node (bb, sl) lives at [sl, bb, :]).
AllGathers are split into two half-collectives (blocks 0-48 / 49-97) so
each half overlaps compute; gather tables are the two half-outputs, each
addressed through 2 int16 windows (cores 0-3 / 4-7).

  P1: h1' = dinv .* (x_pos @ W1) -> resident h1p + ag1_inA/B stores
  AG1a (after group 6), AG1b (at end of P1)
  P3: per super (7 dst blocks): 4 dma_gather (one per (half, window)),
      is_equal indicator per (super, class), chunk matmuls accumulate in
      7 psum banks; self-loop via identity matmul from resident h1p;
      then per block: dinv scale -> relu+b1 -> @W2 -> dinv scale -> h2'
      (resident + stores to ag2_inA/B)
  AG2a (after super 6), AG2b (end of P3)
  P5: same gathers; chunk matmuls [dst, CPAD]; self-loop via resident
      h2' VE-add; dinv scale + b2 -> batched store to out_s
Host: unpermute rows, slice [:N0, :CLS].
"""

from dataclasses import dataclass

import numpy as np

import concourse.bass as bass
import concourse.mybir as mybir
import concourse.tile as tile
from concourse import bacc

FP = mybir.dt.float32
BF = mybir.dt.bfloat16


@dataclass
class Cfg:
    N0: int = 100000     # real nodes
    W: int = 8           # cores
    SHARD: int = 12544   # nodes per core (98 * 128)
    F: int = 512         # in features
    HID: int = 128
    CLS: int = 40
    CPAD: int = 128
    NC: int = 4          # gather classes (half, core-window)
    SPB: int = 7         # dst blocks per super
    GT: int = 7          # P1 col-tiles per load group

    @property
    def NP(self):
        return self.W * self.SHARD

    @property
    def NB(self):
        return self.SHARD // 128

    @property
    def BH(self):
        return self.NB // 2  # blocks per half (49)

    @property
    def RH(self):
        return 128 * self.BH  # rows per core per half table (6272)

    @property
    def WS(self):
        return 4 * self.RH   # rows per int16 window (25088)

    @property
    def NS(self):
        return self.NB // self.SPB


@dataclass
class Meta:
    kq1: np.ndarray = None   # [NB, NC] chunks per (block, class)
    kq2: np.ndarray = None
    node_of_pos: np.ndarray = None  # [W, SHARD] -> node id


def _route_edges(cfg, cls, lidx, dst_pos):
    """Bucket edges by (core of dst, super, class of src, block).
    Chunk stream order per core: (super, class, bb in super, chunk).
    Returns idx16 [W,128,CT*8], dl bf16 [W,128,CT], kq [NB,NC]."""
    import ml_dtypes
    W, SHARD, NC = cfg.W, cfg.SHARD, cfg.NC
    NB, SPB, NS = cfg.NB, cfg.SPB, cfg.NS

    c = dst_pos // SHARD
    r = dst_pos % SHARD
    bb = r // 128
    sl = r % 128

    key = (c * NB + bb) * NC + cls
    order = np.argsort(key, kind="stable")
    lidx_s = lidx[order]
    sl_s = sl[order]

    nseg = W * NB * NC
    counts = np.bincount(key[order], minlength=nseg).reshape(W, NB, NC)
    kq = np.maximum(
        np.ceil(counts.max(axis=0) / 128).astype(np.int64), 1)  # [NB, NC]

    # stream offset of (bb, class) segment: order (s, class, bb in s)
    seg_off = np.zeros((NB, NC), dtype=np.int64)
    cursor = 0
    for s in range(NS):
        for qi in range(NC):
            for j in range(SPB):
                b = s * SPB + j
                seg_off[b, qi] = cursor
                cursor += kq[b, qi] * 128
    CT = int(kq.sum())
    EPAD = CT * 128
    assert cursor == EPAD

    seg_start = np.zeros(nseg + 1, dtype=np.int64)
    seg_start[1:] = np.cumsum(counts.reshape(-1))

    idx16 = np.zeros((W, 128, CT * 8), dtype=np.int16)
    dl16 = np.zeros((W, 128, CT), dtype=ml_dtypes.bfloat16)
    for ci in range(W):
        idx_pad = np.zeros(EPAD, dtype=np.int64)
        dl_pad = np.full(EPAD, -1.0, dtype=np.float32)
        for bi in range(NB):
            for qi in range(NC):
                sidx = (ci * NB + bi) * NC + qi
                s0, s1 = seg_start[sidx], seg_start[sidx + 1]
                n = s1 - s0
                if n == 0:
                    continue
                o = seg_off[bi, qi]
                idx_pad[o:o + n] = lidx_s[s0:s1]
                dl_pad[o:o + n] = sl_s[s0:s1]
        a = idx_pad.reshape(CT, 8, 16)
        wrapped = a.transpose(2, 0, 1).reshape(16, CT * 8)
        idx16[ci] = np.tile(wrapped, (8, 1)).astype(np.int16)
        dl16[ci] = dl_pad.reshape(CT, 128).T.astype(ml_dtypes.bfloat16)
    return idx16, dl16, kq


def preprocess(cfg: Cfg, x, edge_index, W1, b1, W2, b2):
    import ml_dtypes
    N0, W, SHARD, NP = cfg.N0, cfg.W, cfg.SHARD, cfg.NP
    NB = cfg.NB
    x = np.asarray(x, dtype=np.float32)
    edge_index = np.asarray(edge_index)
    W1 = np.asarray(W1, np.float32)
    b1 = np.asarray(b1, np.float32)
    W2 = np.asarray(W2, np.float32)
    b2 = np.asarray(b2, np.float32)

    s = edge_index[0].astype(np.int64)
    d = edge_index[1].astype(np.int64)
    loops = np.arange(N0, dtype=np.int64)
    d_all = np.concatenate([d, loops])

    deg = np.bincount(d_all, minlength=NP).astype(np.float64)
    with np.errstate(divide="ignore"):
        dinv = np.where(deg > 0, 1.0 / np.sqrt(deg), 0.0).astype(np.float32)

    # degree-balanced serpentine block assignment over all W*NB blocks
    nblocks = W * NB
    order = np.argsort(-deg[:N0], kind="stable")
    all_ids = np.concatenate([order, np.arange(N0, NP, dtype=np.int64)])
    rr = np.arange(NP, dtype=np.int64)
    cyc = rr % (2 * nblocks)
    blk = np.where(cyc < nblocks, cyc, 2 * nblocks - 1 - cyc)
    slot_ctr = rr // (2 * nblocks) * 2 + (cyc >= nblocks).astype(np.int64)
    pos = (blk % W) * SHARD + (blk // W) * 128 + slot_ctr
    pos_of_node = np.empty(NP, dtype=np.int64)
    pos_of_node[all_ids] = pos
    node_of_pos = np.empty(NP, dtype=np.int64)
    node_of_pos[pos] = all_ids

    def gcls(p):
        """(class, lidx) of src pos: class = (bb//BH)*2 + c//4,
        lidx = (c%4)*RH + sl*BH + bb%BH."""
        BH, RH = cfg.BH, cfg.RH
        c = p // SHARD
        r = p % SHARD
        bb = r // 128
        sl = r % 128
        cls = (bb // BH) * 2 + c // 4
        lidx = (c % 4) * RH + sl * BH + (bb % BH)
        return cls, lidx

    # both layers exclude self-loops (handled on-device from residents)
    src_pos = pos_of_node[s]
    dst_pos = pos_of_node[d]
    cls, lidx = gcls(src_pos)
    idx16_1, dl1, kq1 = _route_edges(cfg, cls, lidx, dst_pos)
    idx16_2, dl2, kq2 = idx16_1, dl1, kq1

    dinv_pos = dinv[node_of_pos.reshape(W, SHARD)]  # [W, SHARD]

    xpad = np.zeros((NP, cfg.F), np.float32)
    xpad[:N0] = x
    ident = np.eye(128, dtype=np.float32)
    per_core = []
    for c in range(W):
        xs = xpad[node_of_pos[c * SHARD:(c + 1) * SHARD]]  # [SHARD, F] pos
        dpc = dinv_pos[c]
        inp = {
            "xT": np.ascontiguousarray(xs.T).astype(ml_dtypes.bfloat16),
            "w1": W1.astype(ml_dtypes.bfloat16),
            "b1col": b1.reshape(cfg.HID, 1).copy(),
            "w2p": np.pad(W2, ((0, 0), (0, cfg.CPAD - cfg.CLS))
                          ).astype(ml_dtypes.bfloat16),
            "b2rep": np.broadcast_to(
                np.pad(b2, (0, cfg.CPAD - cfg.CLS)), (128, cfg.CPAD)).copy(),
            "iota": np.broadcast_to(
                np.arange(128, dtype=np.float32),
                (128, 128)).astype(ml_dtypes.bfloat16),
            "ident": ident.astype(ml_dtypes.bfloat16),
            "idx1": idx16_1[c], "dl1": dl1[c],
            "dinv_pcT": np.ascontiguousarray(
                dpc.reshape(NB, 128).T).copy(),                 # [128, NB]
            "dinv_pr": np.broadcast_to(
                dpc, (128, SHARD)).astype(ml_dtypes.bfloat16),  # [128, SHARD]
        }
        per_core.append(inp)

    meta = Meta(kq1=kq1, kq2=kq2, node_of_pos=node_of_pos.reshape(W, SHARD))
    return per_core, meta, dinv


def postprocess(cfg: Cfg, outs, meta: Meta):
    """outs: list of [128, NB, CPAD] per core -> [N0, CLS] node order."""
    res = np.zeros((cfg.NP, cfg.CPAD), np.float32)
    for c in range(cfg.W):
        blockmaj = np.transpose(outs[c], (1, 0, 2)).reshape(
            cfg.SHARD, cfg.CPAD)
        res[meta.node_of_pos[c]] = blockmaj
    return res[:cfg.N0, :cfg.CLS]


def _superplan(cfg, kq):
    """Static per-super chunk layout: co[s], sct[s], nch[s][q], lbo[s][q][b]."""
    NS, SPB, NC = cfg.NS, cfg.SPB, cfg.NC
    co, sct, nch, lbo = [], [], [], []
    cursor = 0
    for s in range(NS):
        co.append(cursor)
        nq, lb = [], []
        loc = 0
        for qi in range(NC):
            lbq = []
            for j in range(SPB):
                lbq.append(loc)
                loc += int(kq[s * SPB + j, qi])
            lb.append(lbq)
            nq.append(sum(int(kq[s * SPB + j, qi]) for j in range(SPB)))
        nch.append(nq)
        lbo.append(lb)
        sct.append(loc)
        cursor += loc
    return co, sct, nch, lbo


def build(cfg: Cfg, meta: Meta):
    W, SHARD, NP, F = cfg.W, cfg.SHARD, cfg.NP, cfg.F
    HID, CPAD, NB, BH, RH, WS = (cfg.HID, cfg.CPAD, cfg.NB, cfg.BH, cfg.RH,
                                 cfg.WS)
    NC, SPB, NS, GT = cfg.NC, cfg.SPB, cfg.NS, cfg.GT
    NH = W * RH
    HGRP = (cfg.NB // cfg.SPB) // 2
    kq1 = meta.kq1
    CT1 = int(kq1.sum())
    KT = F // 128
    GC = GT * 128  # cols per P1 load group
    NG = SHARD // GC

    co1, sct1, nch1, lbo1 = _superplan(cfg, kq1)
    MAXSCT = max(sct1)
    MAXNQ = max(max(r) for r in nch1)
    MAXSCTA = max(n[0] + n[1] for n in nch1)
    MAXSCTB = max(n[2] + n[3] for n in nch1)

    nc = bacc.Bacc("TRN2", target_bir_lowering=False, debug=False,
                   num_devices=W, num_swdge_queues=4)

    xT = nc.dram_tensor("xT", [F, SHARD], BF, kind="ExternalInput")
    w1 = nc.dram_tensor("w1", [F, HID], BF, kind="ExternalInput")
    b1col = nc.dram_tensor("b1col", [HID, 1], FP, kind="ExternalInput")
    w2p = nc.dram_tensor("w2p", [HID, CPAD], BF, kind="ExternalInput")
    b2rep = nc.dram_tensor("b2rep", [128, CPAD], FP, kind="ExternalInput")
    iota = nc.dram_tensor("iota", [128, 128], BF, kind="ExternalInput")
    identt = nc.dram_tensor("ident", [128, 128], BF, kind="ExternalInput")
    idx1 = nc.dram_tensor("idx1", [128, CT1 * 8], mybir.dt.int16,
                          kind="ExternalInput")
    dl1 = nc.dram_tensor("dl1", [128, CT1], BF, kind="ExternalInput")
    dinv_pcT = nc.dram_tensor("dinv_pcT", [128, NB], FP, kind="ExternalInput")
    dinv_pr = nc.dram_tensor("dinv_pr", [128, SHARD], BF, kind="ExternalInput")
    out_s = nc.dram_tensor("out_s", [128, NB, CPAD], FP, kind="ExternalOutput")

    ag1_inA = nc.dram_tensor("ag1_inA", [128, BH, HID], BF)
    ag1_inB = nc.dram_tensor("ag1_inB", [128, BH, HID], BF)
    ag1_outA = nc.dram_tensor("ag1_outA", [NH, HID], BF, addr_space="Shared")
    ag1_outB = nc.dram_tensor("ag1_outB", [NH, HID], BF, addr_space="Shared")
    ag2_inA = nc.dram_tensor("ag2_inA", [128, BH, CPAD], BF)
    ag2_inB = nc.dram_tensor("ag2_inB", [128, BH, CPAD], BF)
    ag2_outA = nc.dram_tensor("ag2_outA", [NH, CPAD], BF, addr_space="Shared")
    ag2_outB = nc.dram_tensor("ag2_outB", [NH, CPAD], BF, addr_space="Shared")

    def ag(ins_ap, outs_ap):
        nc.gpsimd.collective_compute(
            "AllGather", mybir.AluOpType.bypass,
            replica_groups=[list(range(W))],
            ins=[ins_ap], outs=[outs_ap],
        )

    with tile.TileContext(nc) as tc:
        with (
            tc.tile_pool(name="const", bufs=1) as cpool,
            tc.tile_pool(name="p1x", bufs=2) as p1pool,
            tc.tile_pool(name="meta1", bufs=2) as mpool,
            tc.tile_pool(name="gathA", bufs=3) as gpoolA,
            tc.tile_pool(name="gathB", bufs=2) as gpoolB,
            tc.tile_pool(name="indp", bufs=2) as ipool,
            tc.tile_pool(name="mid", bufs=3) as midpool,
            tc.tile_pool(name="outg", bufs=2) as opool,
            tc.tile_pool(name="ps", bufs=1, space="PSUM") as pspool,
            tc.tile_pool(name="psw", bufs=1, space="PSUM") as pswpool,
        ):
            # ---- constants ----
            iota_t = cpool.tile([128, 128], BF)
            nc.sync.dma_start(out=iota_t[:, :], in_=iota[:, :])
            ident_t = cpool.tile([128, 128], BF)
            nc.sync.dma_start(out=ident_t[:, :], in_=identt[:, :])
            b1_t = cpool.tile([HID, 1], FP)
            nc.sync.dma_start(out=b1_t[:, :], in_=b1col[:, :])
            w2_t = cpool.tile([HID, CPAD], BF)
            nc.sync.dma_start(out=w2_t[:, :], in_=w2p[:, :])
            b2_t = cpool.tile([128, CPAD], FP)
            nc.sync.dma_start(out=b2_t[:, :], in_=b2rep[:, :])
            w1k_t = cpool.tile([128, KT, HID], BF)
            for k in range(KT):
                nc.sync.dma_start(out=w1k_t[:, k, :],
                                  in_=w1[k * 128:(k + 1) * 128, :])
            dpcT_t = cpool.tile([128, NB], FP)
            nc.sync.dma_start(out=dpcT_t[:, :], in_=dinv_pcT[:, :])
            h1p_res = cpool.tile([128, NB, HID], BF)
            h2p_res = cpool.tile([128, NB, CPAD], BF)

            qctr = [0]

            def next_q():
                qctr[0] = (qctr[0] + 1) % 4
                return qctr[0]

            # ---- P1: h1' = dinv .* (x @ W1), streamed col groups ----
            for g in range(NG):
                xt = p1pool.tile([128, KT, GC], BF, tag="xt")
                for k in range(KT):
                    nc.sync.dma_start(
                        out=xt[:, k, :],
                        in_=xT[k * 128:(k + 1) * 128, g * GC:(g + 1) * GC])
                for t in range(GT):
                    blk = g * GT + t
                    psh = pspool.tile([128, HID], FP, space="PSUM",
                                      tag=f"acc{t}", name=f"acc{t}")
                    for k in range(KT):
                        nc.tensor.matmul(
                            out=psh[:, :],
                            lhsT=xt[:, k, t * 128:(t + 1) * 128],
                            rhs=w1k_t[:, k, :],
                            start=(k == 0), stop=(k == KT - 1))
                    nc.scalar.activation(
                        out=h1p_res[:, blk, :], in_=psh[:, :],
                        func=mybir.ActivationFunctionType.Copy,
                        scale=dpcT_t[:, blk:blk + 1])
                if g < HGRP:
                    nc.sync.dma_start(
                        out=ag1_inA[:, g * GT:(g + 1) * GT, :],
                        in_=h1p_res[:, g * GT:(g + 1) * GT, :])
                else:
                    nc.sync.dma_start(
                        out=ag1_inB[:, (g - HGRP) * GT:(g - HGRP + 1) * GT, :],
                        in_=h1p_res[:, g * GT:(g + 1) * GT, :])
                if g == HGRP - 1:
                    ag(ag1_inA[:, :, :], ag1_outA[:, :])
            ag(ag1_inB[:, :, :], ag1_outB[:, :])

            # ---- P3: L1 aggregation + relu + @W2 -> h2' ----
            for s in range(NS):
                sct = sct1[s]
                o0 = co1[s]
                ixt = mpool.tile([128, MAXSCT * 8], mybir.dt.int16, tag="ix")
                nc.sync.dma_start(out=ixt[:, :sct * 8],
                                  in_=idx1[:, o0 * 8:(o0 + sct) * 8])
                dlt = mpool.tile([128, MAXSCT], BF, tag="dl")
                nc.sync.dma_start(out=dlt[:, :sct], in_=dl1[:, o0:o0 + sct])

                gbufA = gpoolA.tile([128, MAXSCTA, HID], BF, tag="ga")
                gbufB = gpoolB.tile([128, MAXSCTB, HID], BF, tag="gb")
                scta = nch1[s][0] + nch1[s][1]
                lq = 0
                for qi in range(NC):
                    nch = nch1[s][qi]
                    tbl = ag1_outA if qi < 2 else ag1_outB
                    gb = gbufA if qi < 2 else gbufB
                    lo = lq if qi < 2 else lq - scta
                    nc.gpsimd.dma_gather(
                        gb[:, lo:lo + nch, :],
                        tbl[(qi % 2) * WS:(qi % 2 + 1) * WS, :],
                        ixt[:, lq * 8:(lq + nch) * 8],
                        nch * 128, nch * 128, HID,
                        single_packet=False, queue_num=next_q(),
                    )
                    lq += nch

                accs = [pspool.tile([128, 128], FP, space="PSUM",
                                    tag=f"acc{j}", name=f"acc{j}")
                        for j in range(SPB)]
                # self-loop: ps1 := h1p_blk^T via identity matmul
                for j in range(SPB):
                    bb = s * SPB + j
                    nc.tensor.matmul(out=accs[j][:, :],
                                     lhsT=h1p_res[:, bb, :],
                                     rhs=ident_t[:, :],
                                     start=True, stop=False)
                for qi in range(NC):
                    nch = nch1[s][qi]
                    lq = lbo1[s][qi][0]
                    ind = ipool.tile([128, MAXNQ, 128], BF, tag="i")
                    nc.vector.tensor_tensor(
                        out=ind[:, :nch, :],
                        in0=dlt[:, lq:lq + nch].to_broadcast([128, nch, 128]),
                        in1=iota_t[:, None, :].to_broadcast([128, nch, 128]),
                        op=mybir.AluOpType.is_equal,
                    )
                    for j in range(SPB):
                        bb = s * SPB + j
                        kq = int(kq1[bb, qi])
                        ps1 = accs[j]
                        boff = lbo1[s][qi][j]
                        gb = gbufA if qi < 2 else gbufB
                        go = 0 if qi < 2 else scta
                        for k in range(kq):
                            ck = boff + k
                            nc.tensor.matmul(
                                out=ps1[:, :],
                                lhsT=gb[:, ck - go, :],
                                rhs=ind[:, ck - lq, :],
                                start=False,
                                stop=(qi == NC - 1 and k == kq - 1))

                dprs = mpool.tile([128, SPB * 128], BF, tag="dprs")
                nc.sync.dma_start(
                    out=dprs[:, :],
                    in_=dinv_pr[:, s * SPB * 128:(s + 1) * SPB * 128])
                for j in range(SPB):
                    bb = s * SPB + j
                    ps1 = accs[j]
                    t1 = midpool.tile([128, 128], FP, tag="t1")
                    nc.vector.tensor_tensor(
                        out=t1[:, :], in0=ps1[:, :],
                        in1=dprs[:, j * 128:(j + 1) * 128],
                        op=mybir.AluOpType.mult)
                    r1 = midpool.tile([128, 128], BF, tag="r1")
                    nc.scalar.activation(
                        out=r1[:, :], in_=t1[:, :],
                        func=mybir.ActivationFunctionType.Relu,
                        bias=b1_t[:, :1])
                    ps2 = pswpool.tile([128, CPAD], FP, space="PSUM",
                                       tag="accw")
                    nc.tensor.matmul(out=ps2[:, :], lhsT=r1[:, :],
                                     rhs=w2_t[:, :], start=True, stop=True)
                    nc.scalar.activation(
                        out=h2p_res[:, bb, :], in_=ps2[:, :],
                        func=mybir.ActivationFunctionType.Copy,
                        scale=dpcT_t[:, bb:bb + 1])
                if s < HGRP:
                    nc.sync.dma_start(
                        out=ag2_inA[:, s * SPB:(s + 1) * SPB, :],
                        in_=h2p_res[:, s * SPB:(s + 1) * SPB, :])
                else:
                    sb = s - HGRP
                    nc.sync.dma_start(
                        out=ag2_inB[:, sb * SPB:(sb + 1) * SPB, :],
                        in_=h2p_res[:, s * SPB:(s + 1) * SPB, :])
                if s == HGRP - 1:
                    ag(ag2_inA[:, :, :], ag2_outA[:, :])
            ag(ag2_inB[:, :, :], ag2_outB[:, :])

            # ---- P5: L2 aggregation + self-loop + b2 -> out ----
            for s in range(NS):
                sct = sct1[s]
                o0 = co1[s]
                ixt = mpool.tile([128, MAXSCT * 8], mybir.dt.int16, tag="ix")
                nc.sync.dma_start(out=ixt[:, :sct * 8],
                                  in_=idx1[:, o0 * 8:(o0 + sct) * 8])
                dlt = mpool.tile([128, MAXSCT], BF, tag="dl")
                nc.sync.dma_start(out=dlt[:, :sct], in_=dl1[:, o0:o0 + sct])

                gbufA = gpoolA.tile([128, MAXSCTA, CPAD], BF, tag="ga")
                gbufB = gpoolB.tile([128, MAXSCTB, CPAD], BF, tag="gb")
                scta = nch1[s][0] + nch1[s][1]
                lq = 0
                for qi in range(NC):
                    nch = nch1[s][qi]
                    tbl = ag2_outA if qi < 2 else ag2_outB
                    gb = gbufA if qi < 2 else gbufB
                    lo = lq if qi < 2 else lq - scta
                    nc.gpsimd.dma_gather(
                        gb[:, lo:lo + nch, :],
                        tbl[(qi % 2) * WS:(qi % 2 + 1) * WS, :],
                        ixt[:, lq * 8:(lq + nch) * 8],
                        nch * 128, nch * 128, CPAD,
                        single_packet=False, queue_num=next_q(),
                    )
                    lq += nch

                accs = [pspool.tile([128, CPAD], FP, space="PSUM",
                                    tag=f"acc{j}", name=f"acc{j}")
                        for j in range(SPB)]
                for j in range(SPB):
                    bb = s * SPB + j
                    nc.tensor.matmul(out=accs[j][:, :],
                                     lhsT=ident_t[:, :],
                                     rhs=h2p_res[:, bb, :],
                                     start=True, stop=False)
                for qi in range(NC):
                    nch = nch1[s][qi]
                    lq = lbo1[s][qi][0]
                    ind = ipool.tile([128, MAXNQ, 128], BF, tag="i")
                    nc.vector.tensor_tensor(
                        out=ind[:, :nch, :],
                        in0=dlt[:, lq:lq + nch].to_broadcast([128, nch, 128]),
                        in1=iota_t[:, None, :].to_broadcast([128, nch, 128]),
                        op=mybir.AluOpType.is_equal,
                    )
                    for j in range(SPB):
                        bb = s * SPB + j
                        kq = int(kq1[bb, qi])
                        ps3 = accs[j]
                        boff = lbo1[s][qi][j]
                        gb = gbufA if qi < 2 else gbufB
                        go = 0 if qi < 2 else scta
                        for k in range(kq):
                            ck = boff + k
                            nc.tensor.matmul(
                                out=ps3[:, :],
                                lhsT=ind[:, ck - lq, :],
                                rhs=gb[:, ck - go, :],
                                start=False,
                                stop=(qi == NC - 1 and k == kq - 1))

                og = opool.tile([128, SPB, CPAD], FP, tag="og")
                for j in range(SPB):
                    bb = s * SPB + j
                    ps3 = accs[j]
                    v = midpool.tile([128, CPAD], FP, tag="v")
                    nc.scalar.activation(
                        out=v[:, :], in_=ps3[:, :],
                        func=mybir.ActivationFunctionType.Copy,
                        scale=dpcT_t[:, bb:bb + 1])
                    nc.vector.tensor_tensor(
                        out=og[:, j, :], in0=v[:, :], in1=b2_t[:, :],
                        op=mybir.AluOpType.add)
                nc.sync.dma_start(out=out_s[:, s * SPB:(s + 1) * SPB, :],
                                  in_=og[:, :, :])

    nc.compile()
    return nc


# ======================================================================
# kernel() entry point
# ======================================================================
import os as _os


LAST_EXEC_NS = None
LAST_RES = None


def kernel(x, edge_index, W1, b1, W2, b2):
    """Full-input GCN kernel: shards across 8 NeuronCores internally."""
    global LAST_EXEC_NS, LAST_RES
    import numpy as _np

    trace = bool(int(_os.environ.get("GCN_TRACE", "0")))
    if trace:
        try:
            import sys as _sys
            import types as _types
            from trn_agent_boot.trn_boot import _ntff_profile_via_ctypes
            if "antenv.axon_hooks" not in _sys.modules:
                _hook = _ntff_profile_via_ctypes("/opt/axon/libaxon_pjrt.so")
                _m = _types.ModuleType("antenv.axon_hooks")
                _m.get_axon_ntff_profile_hook = lambda: _hook
                _m.set_axon_ntff_profile_hook = lambda h: None
                _sys.modules["antenv.axon_hooks"] = _m
        except Exception:
            trace = False

    from concourse.bass_utils import run_bass_kernel_spmd

    cfg = Cfg()
    per_core, meta, _ = preprocess(cfg, x, edge_index, W1, b1, W2, b2)
    nc = build(cfg, meta)
    res = run_bass_kernel_spmd(
        nc, per_core, core_ids=list(range(cfg.W)), trace=trace,
    )
    LAST_EXEC_NS = res.exec_time_ns
    LAST_RES = res
    outs = [res.results[c]["out_s"] for c in range(cfg.W)]
    return _np.ascontiguousarray(postprocess(cfg, outs, meta).astype(_np.float32))


# revision 11
# speedup vs baseline: 1.2021x; 1.2021x over previous
"""GCN 2-layer kernel for trn2: host preprocessing + Bass kernel builder.

Math (per GCNConv, PyG-style):
  out = D^-1/2 (A+I) D^-1/2 (X W) + b
Layer1 -> relu -> Layer2.

Device plan (8 cores, SPMD), all tensors in pos-order with p-major DRAM
layout [128, NB, feat] (row of node (bb, sl) lives at [sl, bb, :]).
AllGathers are split into two half-collectives (blocks 0-48 / 49-97) so
each half overlaps compute; gather tables are the two half-outputs, each
addressed through 2 int16 windows (cores 0-3 / 4-7).

  P1: h1' = dinv .* (x_pos @ W1) -> resident h1p + ag1_inA/B stores
  AG1a (after group 6), AG1b (at end of P1)
  P3: per super (7 dst blocks): 4 dma_gather (one per (half, window)),
      is_equal indicator per (super, class), chunk matmuls accumulate in
      7 psum banks; self-loop via identity matmul from resident h1p;
      then per block: dinv scale -> relu+b1 -> @W2 -> dinv scale -> h2'
      (resident + stores to ag2_inA/B)
  AG2a (after super 6), AG2b (end of P3)
  P5: same gathers; chunk matmuls [dst, CPAD]; self-loop via resident
      h2' VE-add; dinv scale + b2 -> batched store to out_s
Host: unpermute rows, slice [:N0, :CLS].
"""

from dataclasses import dataclass

import numpy as np

import concourse.bass as bass
import concourse.mybir as mybir
import concourse.tile as tile
from concourse import bacc

FP = mybir.dt.float32
BF = mybir.dt.bfloat16


@dataclass
class Cfg:
    N0: int = 100000     # real nodes
    W: int = 8           # cores
    SHARD: int = 12544   # nodes per core (98 * 128)
    F: int = 512         # in features
    HID: int = 128
    CLS: int = 40
    CPAD: int = 128
    NC: int = 4          # gather classes (half, core-window)
    SPB: int = 7         # dst blocks per super
    GT: int = 7          # P1 col-tiles per load group

    @property
    def NP(self):
        return self.W * self.SHARD

    @property
    def NB(self):
        return self.SHARD // 128

    @property
    def BH(self):
        return self.NB // 2  # blocks per half (49)

    @property
    def RH(self):
        return 128 * self.BH  # rows per core per half table (6272)

    @property
    def WS(self):
        return 4 * self.RH   # rows per int16 window (25088)

    @property
    def NS(self):
        return self.NB // self.SPB


@dataclass
class Meta:
    kq1: np.ndarray = None   # [NB, NC] chunks per (block, class)
    kq2: np.ndarray = None
    node_of_pos: np.ndarray = None  # [W, SHARD] -> node id


def _route_edges(cfg, cls, lidx, dst_pos):
    """Bucket edges by (core of dst, super, class of src, block).
    Chunk stream order per core: (super, class, bb in super, chunk).
    Returns idx16 [W,128,CT*8], dl bf16 [W,128,CT], kq [NB,NC]."""
    import ml_dtypes
    W, SHARD, NC = cfg.W, cfg.SHARD, cfg.NC
    NB, SPB, NS = cfg.NB, cfg.SPB, cfg.NS

    c = dst_pos // SHARD
    r = dst_pos % SHARD
    bb = r // 128
    sl = r % 128

    key = (c * NB + bb) * NC + cls
    order = np.argsort(key, kind="stable")
    lidx_s = lidx[order]
    sl_s = sl[order]

    nseg = W * NB * NC
    counts = np.bincount(key[order], minlength=nseg).reshape(W, NB, NC)
    kq = np.maximum(
        np.ceil(counts.max(axis=0) / 128).astype(np.int64), 1)  # [NB, NC]

    # stream offset of (bb, class) segment: order (s, class, bb in s)
    seg_off = np.zeros((NB, NC), dtype=np.int64)
    cursor = 0
    for s in range(NS):
        for qi in range(NC):
            for j in range(SPB):
                b = s * SPB + j
                seg_off[b, qi] = cursor
                cursor += kq[b, qi] * 128
    CT = int(kq.sum())
    EPAD = CT * 128
    assert cursor == EPAD

    seg_start = np.zeros(nseg + 1, dtype=np.int64)
    seg_start[1:] = np.cumsum(counts.reshape(-1))

    idx16 = np.zeros((W, 128, CT * 8), dtype=np.int16)
    dl16 = np.zeros((W, 128, CT), dtype=ml_dtypes.bfloat16)
    for ci in range(W):
        idx_pad = np.zeros(EPAD, dtype=np.int64)
        dl_pad = np.full(EPAD, -1.0, dtype=np.float32)
        for bi in range(NB):
            for qi in range(NC):
                sidx = (ci * NB + bi) * NC + qi
                s0, s1 = seg_start[sidx], seg_start[sidx + 1]
                n = s1 - s0
                if n == 0:
                    continue
                o = seg_off[bi, qi]
                idx_pad[o:o + n] = lidx_s[s0:s1]
                dl_pad[o:o + n] = sl_s[s0:s1]
        a = idx_pad.reshape(CT, 8, 16)
        wrapped = a.transpose(2, 0, 1).reshape(16, CT * 8)
        idx16[ci] = np.tile(wrapped, (8, 1)).astype(np.int16)
        dl16[ci] = dl_pad.reshape(CT, 128).T.astype(ml_dtypes.bfloat16)
    return idx16, dl16, kq


def preprocess(cfg: Cfg, x, edge_index, W1, b1, W2, b2):
    import ml_dtypes
    N0, W, SHARD, NP = cfg.N0, cfg.W, cfg.SHARD, cfg.NP
    NB = cfg.NB
    x = np.asarray(x, dtype=np.float32)
    edge_index = np.asarray(edge_index)
    W1 = np.asarray(W1, np.float32)
    b1 = np.asarray(b1, np.float32)
    W2 = np.asarray(W2, np.float32)
    b2 = np.asarray(b2, np.float32)

    s = edge_index[0].astype(np.int64)
    d = edge_index[1].astype(np.int64)
    loops = np.arange(N0, dtype=np.int64)
    d_all = np.concatenate([d, loops])

    deg = np.bincount(d_all, minlength=NP).astype(np.float64)
    with np.errstate(divide="ignore"):
        dinv = np.where(deg > 0, 1.0 / np.sqrt(deg), 0.0).astype(np.float32)

    # degree-balanced serpentine block assignment over all W*NB blocks
    nblocks = W * NB
    order = np.argsort(-deg[:N0], kind="stable")
    all_ids = np.concatenate([order, np.arange(N0, NP, dtype=np.int64)])
    rr = np.arange(NP, dtype=np.int64)
    cyc = rr % (2 * nblocks)
    blk = np.where(cyc < nblocks, cyc, 2 * nblocks - 1 - cyc)
    slot_ctr = rr // (2 * nblocks) * 2 + (cyc >= nblocks).astype(np.int64)
    pos = (blk % W) * SHARD + (blk // W) * 128 + slot_ctr
    pos_of_node = np.empty(NP, dtype=np.int64)
    pos_of_node[all_ids] = pos
    node_of_pos = np.empty(NP, dtype=np.int64)
    node_of_pos[pos] = all_ids

    def gcls(p):
        """(class, lidx) of src pos: class = (bb//BH)*2 + c//4,
        lidx = (c%4)*RH + sl*BH + bb%BH."""
        BH, RH = cfg.BH, cfg.RH
        c = p // SHARD
        r = p % SHARD
        bb = r // 128
        sl = r % 128
        cls = (bb // BH) * 2 + c // 4
        lidx = (c % 4) * RH + sl * BH + (bb % BH)
        return cls, lidx

    # both layers exclude self-loops (handled on-device from residents)
    src_pos = pos_of_node[s]
    dst_pos = pos_of_node[d]
    cls, lidx = gcls(src_pos)
    idx16_1, dl1, kq1 = _route_edges(cfg, cls, lidx, dst_pos)
    idx16_2, dl2, kq2 = idx16_1, dl1, kq1

    dinv_pos = dinv[node_of_pos.reshape(W, SHARD)]  # [W, SHARD]

    xpad = np.zeros((NP, cfg.F), np.float32)
    xpad[:N0] = x
    ident = np.eye(128, dtype=np.float32)
    per_core = []
    for c in range(W):
        xs = xpad[node_of_pos[c * SHARD:(c + 1) * SHARD]]  # [SHARD, F] pos
        dpc = dinv_pos[c]
        inp = {
            "xT": np.ascontiguousarray(xs.T).astype(ml_dtypes.bfloat16),
            "w1": W1.astype(ml_dtypes.bfloat16),
            "b1col": b1.reshape(cfg.HID, 1).copy(),
            "w2p": np.pad(W2, ((0, 0), (0, cfg.CPAD - cfg.CLS))
                          ).astype(ml_dtypes.bfloat16),
            "b2rep": np.broadcast_to(
                np.pad(b2, (0, cfg.CPAD - cfg.CLS)), (128, cfg.CPAD)).copy(),
            "iota": np.broadcast_to(
                np.arange(128, dtype=np.float32),
                (128, 128)).astype(ml_dtypes.bfloat16),
            "ident": ident.astype(ml_dtypes.bfloat16),
            "idx1": idx16_1[c], "dl1": dl1[c],
            "dinv_pcT": np.ascontiguousarray(
                dpc.reshape(NB, 128).T).copy(),                 # [128, NB]
            "dinv_pr": np.broadcast_to(
                dpc, (128, SHARD)).astype(ml_dtypes.bfloat16),  # [128, SHARD]
        }
        per_core.append(inp)

    meta = Meta(kq1=kq1, kq2=kq2, node_of_pos=node_of_pos.reshape(W, SHARD))
    return per_core, meta, dinv


def postprocess(cfg: Cfg, outs, meta: Meta):
    """outs: list of [128, NB, CPAD] per core -> [N0, CLS] node order."""
    res = np.zeros((cfg.NP, cfg.CPAD), np.float32)
    for c in range(cfg.W):
        blockmaj = np.transpose(outs[c], (1, 0, 2)).reshape(
            cfg.SHARD, cfg.CPAD)
        res[meta.node_of_pos[c]] = blockmaj
    return res[:cfg.N0, :cfg.CLS]


def _superplan(cfg, kq):
    """Static per-super chunk layout: co[s], sct[s], nch[s][q], lbo[s][q][b]."""
    NS, SPB, NC = cfg.NS, cfg.SPB, cfg.NC
    co, sct, nch, lbo = [], [], [], []
    cursor = 0
    for s in range(NS):
        co.append(cursor)
        nq, lb = [], []
        loc = 0
        for qi in range(NC):
            lbq = []
            for j in range(SPB):
                lbq.append(loc)
                loc += int(kq[s * SPB + j, qi])
            lb.append(lbq)
            nq.append(sum(int(kq[s * SPB + j, qi]) for j in range(SPB)))
        nch.append(nq)
        lbo.append(lb)
        sct.append(loc)
        cursor += loc
    return co, sct, nch, lbo


def build(cfg: Cfg, meta: Meta):
    W, SHARD, NP, F = cfg.W, cfg.SHARD, cfg.NP, cfg.F
    HID, CPAD, NB, BH, RH, WS = (cfg.HID, cfg.CPAD, cfg.NB, cfg.BH, cfg.RH,
                                 cfg.WS)
    NC, SPB, NS, GT = cfg.NC, cfg.SPB, cfg.NS, cfg.GT
    kq1 = meta.kq1
    CT1 = int(kq1.sum())
    KT = F // 128
    GC = GT * 128  # cols per P1 load group
    NG = SHARD // GC
    NH = W * RH    # rows per half table
    HGRP = NS // 2  # supers per half (7)
    NH = W * RH    # rows per half table
    HGRP = NS // 2  # supers per half (7)

    co1, sct1, nch1, lbo1 = _superplan(cfg, kq1)
    MAXSCT = max(sct1)
    MAXNQ = max(max(r) for r in nch1)

    nc = bacc.Bacc("TRN2", target_bir_lowering=False, debug=False,
                   num_devices=W, num_swdge_queues=4)

    xT = nc.dram_tensor("xT", [F, SHARD], BF, kind="ExternalInput")
    w1 = nc.dram_tensor("w1", [F, HID], BF, kind="ExternalInput")
    b1col = nc.dram_tensor("b1col", [HID, 1], FP, kind="ExternalInput")
    w2p = nc.dram_tensor("w2p", [HID, CPAD], BF, kind="ExternalInput")
    b2rep = nc.dram_tensor("b2rep", [128, CPAD], FP, kind="ExternalInput")
    iota = nc.dram_tensor("iota", [128, 128], BF, kind="ExternalInput")
    identt = nc.dram_tensor("ident", [128, 128], BF, kind="ExternalInput")
    idx1 = nc.dram_tensor("idx1", [128, CT1 * 8], mybir.dt.int16,
                          kind="ExternalInput")
    dl1 = nc.dram_tensor("dl1", [128, CT1], BF, kind="ExternalInput")
    dinv_pcT = nc.dram_tensor("dinv_pcT", [128, NB], FP, kind="ExternalInput")
    dinv_pr = nc.dram_tensor("dinv_pr", [128, SHARD], BF, kind="ExternalInput")
    out_s = nc.dram_tensor("out_s", [128, NB, CPAD], FP, kind="ExternalOutput")

    ag1_inA = nc.dram_tensor("ag1_inA", [128, BH, HID], BF)
    ag1_inB = nc.dram_tensor("ag1_inB", [128, BH, HID], BF)
    ag1_outA = nc.dram_tensor("ag1_outA", [NH, HID], BF, addr_space="Shared")
    ag1_outB = nc.dram_tensor("ag1_outB", [NH, HID], BF, addr_space="Shared")
    ag2_inA = nc.dram_tensor("ag2_inA", [128, BH, CPAD], BF)
    ag2_inB = nc.dram_tensor("ag2_inB", [128, BH, CPAD], BF)
    ag2_outA = nc.dram_tensor("ag2_outA", [NH, CPAD], BF, addr_space="Shared")
    ag2_outB = nc.dram_tensor("ag2_outB", [NH, CPAD], BF, addr_space="Shared")

    def ag(ins_ap, outs_ap):
        nc.gpsimd.collective_compute(
            "AllGather", mybir.AluOpType.bypass,
            replica_groups=[list(range(W))],
            ins=[ins_ap], outs=[outs_ap],
        )

    with tile.TileContext(nc) as tc:
        with (
            tc.tile_pool(name="const", bufs=1) as cpool,
            tc.tile_pool(name="p1x", bufs=2) as p1pool,
            tc.tile_pool(name="meta1", bufs=2) as mpool,
            tc.tile_pool(name="gath", bufs=2) as gpool,
            tc.tile_pool(name="indp", bufs=2) as ipool,
            tc.tile_pool(name="mid", bufs=3) as midpool,
            tc.tile_pool(name="outg", bufs=2) as opool,
            tc.tile_pool(name="ps", bufs=1, space="PSUM") as pspool,
            tc.tile_pool(name="psw", bufs=1, space="PSUM") as pswpool,
        ):
            # ---- constants ----
            iota_t = cpool.tile([128, 128], BF)
            nc.sync.dma_start(out=iota_t[:, :], in_=iota[:, :])
            ident_t = cpool.tile([128, 128], BF)
            nc.sync.dma_start(out=ident_t[:, :], in_=identt[:, :])
            b1_t = cpool.tile([HID, 1], FP)
            nc.sync.dma_start(out=b1_t[:, :], in_=b1col[:, :])
            w2_t = cpool.tile([HID, CPAD], BF)
            nc.sync.dma_start(out=w2_t[:, :], in_=w2p[:, :])
            b2_t = cpool.tile([128, CPAD], FP)
            nc.sync.dma_start(out=b2_t[:, :], in_=b2rep[:, :])
            w1k_t = cpool.tile([128, KT, HID], BF)
            for k in range(KT):
                nc.sync.dma_start(out=w1k_t[:, k, :],
                                  in_=w1[k * 128:(k + 1) * 128, :])
            dpcT_t = cpool.tile([128, NB], FP)
            nc.sync.dma_start(out=dpcT_t[:, :], in_=dinv_pcT[:, :])
            dpr_t = cpool.tile([128, SHARD], BF)
            nc.sync.dma_start(out=dpr_t[:, :], in_=dinv_pr[:, :])
            h1p_res = cpool.tile([128, NB, HID], BF)
            h2p_res = cpool.tile([128, NB, CPAD], BF)

            qctr = [0]

            def next_q():
                qctr[0] = (qctr[0] + 1) % 4
                return qctr[0]

            # ---- P1: h1' = dinv .* (x @ W1), streamed col groups ----
            for g in range(NG):
                xt = p1pool.tile([128, KT, GC], BF, tag="xt")
                for k in range(KT):
                    nc.sync.dma_start(
                        out=xt[:, k, :],
                        in_=xT[k * 128:(k + 1) * 128, g * GC:(g + 1) * GC])
                for t in range(GT):
                    blk = g * GT + t
                    psh = pspool.tile([128, HID], FP, space="PSUM",
                                      tag=f"acc{t}", name=f"acc{t}")
                    for k in range(KT):
                        nc.tensor.matmul(
                            out=psh[:, :],
                            lhsT=xt[:, k, t * 128:(t + 1) * 128],
                            rhs=w1k_t[:, k, :],
                            start=(k == 0), stop=(k == KT - 1))
                    nc.scalar.activation(
                        out=h1p_res[:, blk, :], in_=psh[:, :],
                        func=mybir.ActivationFunctionType.Copy,
                        scale=dpcT_t[:, blk:blk + 1])
                if g < HGRP:
                    nc.sync.dma_start(
                        out=ag1_inA[:, g * GT:(g + 1) * GT, :],
                        in_=h1p_res[:, g * GT:(g + 1) * GT, :])
                else:
                    nc.sync.dma_start(
                        out=ag1_inB[:, (g - HGRP) * GT:(g - HGRP + 1) * GT, :],
                        in_=h1p_res[:, g * GT:(g + 1) * GT, :])
                if g == HGRP - 1:
                    ag(ag1_inA[:, :, :], ag1_outA[:, :])
            ag(ag1_inB[:, :, :], ag1_outB[:, :])

            # ---- P3: L1 aggregation + relu + @W2 -> h2' ----
            for s in range(NS):
                sct = sct1[s]
                o0 = co1[s]
                ixt = mpool.tile([128, MAXSCT * 8], mybir.dt.int16, tag="ix")
                nc.sync.dma_start(out=ixt[:, :sct * 8],
                                  in_=idx1[:, o0 * 8:(o0 + sct) * 8])
                dlt = mpool.tile([128, MAXSCT], BF, tag="dl")
                nc.sync.dma_start(out=dlt[:, :sct], in_=dl1[:, o0:o0 + sct])

                gbuf = gpool.tile([128, MAXSCT, HID], BF, tag="g")
                lq = 0
                for qi in range(NC):
                    nch = nch1[s][qi]
                    tbl = ag1_outA if qi < 2 else ag1_outB
                    nc.gpsimd.dma_gather(
                        gbuf[:, lq:lq + nch, :],
                        tbl[(qi % 2) * WS:(qi % 2 + 1) * WS, :],
                        ixt[:, lq * 8:(lq + nch) * 8],
                        nch * 128, nch * 128, HID,
                        single_packet=False, queue_num=next_q(),
                    )
                    lq += nch

                accs = [pspool.tile([128, 128], FP, space="PSUM",
                                    tag=f"acc{j}", name=f"acc{j}")
                        for j in range(SPB)]
                # self-loop: ps1 := h1p_blk^T via identity matmul
                for j in range(SPB):
                    bb = s * SPB + j
                    nc.tensor.matmul(out=accs[j][:, :],
                                     lhsT=h1p_res[:, bb, :],
                                     rhs=ident_t[:, :],
                                     start=True, stop=False)
                for qi in range(NC):
                    nch = nch1[s][qi]
                    lq = lbo1[s][qi][0]
                    ind = ipool.tile([128, MAXNQ, 128], BF, tag="i")
                    nc.vector.tensor_tensor(
                        out=ind[:, :nch, :],
                        in0=dlt[:, lq:lq + nch].to_broadcast([128, nch, 128]),
                        in1=iota_t[:, None, :].to_broadcast([128, nch, 128]),
                        op=mybir.AluOpType.is_equal,
                    )
                    for j in range(SPB):
                        bb = s * SPB + j
                        kq = int(kq1[bb, qi])
                        ps1 = accs[j]
                        boff = lbo1[s][qi][j]
                        for k in range(kq):
                            ck = boff + k
                            nc.tensor.matmul(
                                out=ps1[:, :],
                                lhsT=gbuf[:, ck, :],
                                rhs=ind[:, ck - lq, :],
                                start=False,
                                stop=(qi == NC - 1 and k == kq - 1))

                for j in range(SPB):
                    bb = s * SPB + j
                    ps1 = accs[j]
                    t1 = midpool.tile([128, 128], FP, tag="t1")
                    nc.vector.tensor_tensor(
                        out=t1[:, :], in0=ps1[:, :],
                        in1=dpr_t[:, bb * 128:(bb + 1) * 128],
                        op=mybir.AluOpType.mult)
                    r1 = midpool.tile([128, 128], BF, tag="r1")
                    nc.scalar.activation(
                        out=r1[:, :], in_=t1[:, :],
                        func=mybir.ActivationFunctionType.Relu,
                        bias=b1_t[:, :1])
                    ps2 = pswpool.tile([128, CPAD], FP, space="PSUM",
                                       tag="accw")
                    nc.tensor.matmul(out=ps2[:, :], lhsT=r1[:, :],
                                     rhs=w2_t[:, :], start=True, stop=True)
                    nc.scalar.activation(
                        out=h2p_res[:, bb, :], in_=ps2[:, :],
                        func=mybir.ActivationFunctionType.Copy,
                        scale=dpcT_t[:, bb:bb + 1])
                if s < HGRP:
                    nc.sync.dma_start(
                        out=ag2_inA[:, s * SPB:(s + 1) * SPB, :],
                        in_=h2p_res[:, s * SPB:(s + 1) * SPB, :])
                else:
                    sb = s - HGRP
                    nc.sync.dma_start(
                        out=ag2_inB[:, sb * SPB:(sb + 1) * SPB, :],
                        in_=h2p_res[:, s * SPB:(s + 1) * SPB, :])
                if s == HGRP - 1:
                    ag(ag2_inA[:, :, :], ag2_outA[:, :])
            ag(ag2_inB[:, :, :], ag2_outB[:, :])

            # ---- P5: L2 aggregation + self-loop + b2 -> out ----
            for s in range(NS):
                sct = sct1[s]
                o0 = co1[s]
                ixt = mpool.tile([128, MAXSCT * 8], mybir.dt.int16, tag="ix")
                nc.sync.dma_start(out=ixt[:, :sct * 8],
                                  in_=idx1[:, o0 * 8:(o0 + sct) * 8])
                dlt = mpool.tile([128, MAXSCT], BF, tag="dl")
                nc.sync.dma_start(out=dlt[:, :sct], in_=dl1[:, o0:o0 + sct])

                gbuf = gpool.tile([128, MAXSCT, CPAD], BF, tag="g")
                lq = 0
                for qi in range(NC):
                    nch = nch1[s][qi]
                    tbl = ag2_outA if qi < 2 else ag2_outB
                    nc.gpsimd.dma_gather(
                        gbuf[:, lq:lq + nch, :],
                        tbl[(qi % 2) * WS:(qi % 2 + 1) * WS, :],
                        ixt[:, lq * 8:(lq + nch) * 8],
                        nch * 128, nch * 128, CPAD,
                        single_packet=False, queue_num=next_q(),
                    )
                    lq += nch

                accs = [pspool.tile([128, CPAD], FP, space="PSUM",
                                    tag=f"acc{j}", name=f"acc{j}")
                        for j in range(SPB)]
                for j in range(SPB):
                    bb = s * SPB + j
                    nc.tensor.matmul(out=accs[j][:, :],
                                     lhsT=ident_t[:, :],
                                     rhs=h2p_res[:, bb, :],
                                     start=True, stop=False)
                for qi in range(NC):
                    nch = nch1[s][qi]
                    lq = lbo1[s][qi][0]
                    ind = ipool.tile([128, MAXNQ, 128], BF, tag="i")
                    nc.vector.tensor_tensor(
                        out=ind[:, :nch, :],
                        in0=dlt[:, lq:lq + nch].to_broadcast([128, nch, 128]),
                        in1=iota_t[:, None, :].to_broadcast([128, nch, 128]),
                        op=mybir.AluOpType.is_equal,
                    )
                    for j in range(SPB):
                        bb = s * SPB + j
                        kq = int(kq1[bb, qi])
                        ps3 = accs[j]
                        boff = lbo1[s][qi][j]
                        for k in range(kq):
                            ck = boff + k
                            nc.tensor.matmul(
                                out=ps3[:, :],
                                lhsT=ind[:, ck - lq, :],
                                rhs=gbuf[:, ck, :],
                                start=False,
                                stop=(qi == NC - 1 and k == kq - 1))

                og = opool.tile([128, SPB, CPAD], FP, tag="og")
                for j in range(SPB):
                    bb = s * SPB + j
                    ps3 = accs[j]
                    v = midpool.tile([128, CPAD], FP, tag="v")
                    nc.scalar.activation(
                        out=v[:, :], in_=ps3[:, :],
                        func=mybir.ActivationFunctionType.Copy,
                        scale=dpcT_t[:, bb:bb + 1])
                    nc.vector.tensor_tensor(
                        out=og[:, j, :], in0=v[:, :], in1=b2_t[:, :],
                        op=mybir.AluOpType.add)
                nc.sync.dma_start(out=out_s[:, s * SPB:(s + 1) * SPB, :],
                                  in_=og[:, :, :])

    nc.compile()
    return nc


# ======================================================================
# kernel() entry point
# ======================================================================
import os as _os


LAST_EXEC_NS = None
LAST_RES = None


def kernel(x, edge_index, W1, b1, W2, b2):
    """Full-input GCN kernel: shards across 8 NeuronCores internally."""
    global LAST_EXEC_NS, LAST_RES
    import numpy as _np

    trace = bool(int(_os.environ.get("GCN_TRACE", "0")))
    if trace:
        try:
            import sys as _sys
            import types as _types
            from trn_agent_boot.trn_boot import _ntff_profile_via_ctypes
            if "antenv.axon_hooks" not in _sys.modules:
                _hook = _ntff_profile_via_ctypes("/opt/axon/libaxon_pjrt.so")
                _m = _types.ModuleType("antenv.axon_hooks")
                _m.get_axon_ntff_profile_hook = lambda: _hook
                _m.set_axon_ntff_profile_hook = lambda h: None
                _sys.modules["antenv.axon_hooks"] = _m
        except Exception:
            trace = False

    from concourse.bass_utils import run_bass_kernel_spmd

    cfg = Cfg()
    per_core, meta, _ = preprocess(cfg, x, edge_index, W1, b1, W2, b2)
    nc = build(cfg, meta)
    res = run_bass_kernel_spmd(
        nc, per_core, core_ids=list(range(cfg.W)), trace=trace,
    )
    LAST_EXEC_NS = res.exec_time_ns
    LAST_RES = res
    outs = [res.results[c]["out_s"] for c in range(cfg.W)]
    return _np.ascontiguousarray(postprocess(cfg, outs, meta).astype(_np.float32))


# revision 12
# speedup vs baseline: 1.2255x; 1.0195x over previous
"""GCN 2-layer kernel for trn2: host preprocessing + Bass kernel builder.

Math (per GCNConv, PyG-style):
  out = D^-1/2 (A+I) D^-1/2 (X W) + b
Layer1 -> relu -> Layer2.

Device plan (8 cores, SPMD), all tensors in pos-order with p-major DRAM
layout [128, NB, feat] (row of node (bb, sl) lives at [sl, bb, :]).
AllGathers are split into two half-collectives (blocks 0-48 / 49-97) so
each half overlaps compute; gather tables are the two half-outputs, each
addressed through 2 int16 windows (cores 0-3 / 4-7).

  P1: h1' = dinv .* (x_pos @ W1) -> resident h1p + ag1_inA/B stores
  AG1a (after group 6), AG1b (at end of P1)
  P3: per super (7 dst blocks): 4 dma_gather (one per (half, window)),
      is_equal indicator per (super, class), chunk matmuls accumulate in
      7 psum banks; self-loop via identity matmul from resident h1p;
      then per block: dinv scale -> relu+b1 -> @W2 -> dinv scale -> h2'
      (resident + stores to ag2_inA/B)
  AG2a (after super 6), AG2b (end of P3)
  P5: same gathers; chunk matmuls [dst, CPAD]; self-loop via resident
      h2' VE-add; dinv scale + b2 -> batched store to out_s
Host: unpermute rows, slice [:N0, :CLS].
"""

from dataclasses import dataclass

import numpy as np

import concourse.bass as bass
import concourse.mybir as mybir
import concourse.tile as tile
from concourse import bacc

FP = mybir.dt.float32
BF = mybir.dt.bfloat16


@dataclass
class Cfg:
    N0: int = 100000     # real nodes
    W: int = 8           # cores
    SHARD: int = 12544   # nodes per core (98 * 128)
    F: int = 512         # in features
    HID: int = 128
    CLS: int = 40
    CPAD: int = 128
    NC: int = 4          # gather classes (half, core-window)
    SPB: int = 7         # dst blocks per super
    GT: int = 7          # P1 col-tiles per load group

    @property
    def NP(self):
        return self.W * self.SHARD

    @property
    def NB(self):
        return self.SHARD // 128

    @property
    def BH(self):
        return self.NB // 2  # blocks per half (49)

    @property
    def RH(self):
        return 128 * self.BH  # rows per core per half table (6272)

    @property
    def WS(self):
        return 4 * self.RH   # rows per int16 window (25088)

    @property
    def NS(self):
        return self.NB // self.SPB


@dataclass
class Meta:
    kq1: np.ndarray = None   # [NB, NC] chunks per (block, class)
    kq2: np.ndarray = None
    node_of_pos: np.ndarray = None  # [W, SHARD] -> node id


def _route_edges(cfg, cls, lidx, dst_pos):
    """Bucket edges by (core of dst, super, class of src, block).
    Chunk stream order per core: (super, class, bb in super, chunk).
    Returns idx16 [W,128,CT*8], dl bf16 [W,128,CT], kq [NB,NC]."""
    import ml_dtypes
    W, SHARD, NC = cfg.W, cfg.SHARD, cfg.NC
    NB, SPB, NS = cfg.NB, cfg.SPB, cfg.NS

    c = dst_pos // SHARD
    r = dst_pos % SHARD
    bb = r // 128
    sl = r % 128

    key = (c * NB + bb) * NC + cls
    order = np.argsort(key, kind="stable")
    lidx_s = lidx[order]
    sl_s = sl[order]

    nseg = W * NB * NC
    counts = np.bincount(key[order], minlength=nseg).reshape(W, NB, NC)
    kq = np.maximum(
        np.ceil(counts.max(axis=0) / 128).astype(np.int64), 1)  # [NB, NC]

    # stream offset of (bb, class) segment: order (s, class, bb in s)
    seg_off = np.zeros((NB, NC), dtype=np.int64)
    cursor = 0
    for s in range(NS):
        for qi in range(NC):
            for j in range(SPB):
                b = s * SPB + j
                seg_off[b, qi] = cursor
                cursor += kq[b, qi] * 128
    CT = int(kq.sum())
    EPAD = CT * 128
    assert cursor == EPAD

    seg_start = np.zeros(nseg + 1, dtype=np.int64)
    seg_start[1:] = np.cumsum(counts.reshape(-1))

    idx16 = np.zeros((W, 128, CT * 8), dtype=np.int16)
    dl16 = np.zeros((W, 128, CT), dtype=ml_dtypes.bfloat16)
    for ci in range(W):
        idx_pad = np.zeros(EPAD, dtype=np.int64)
        dl_pad = np.full(EPAD, -1.0, dtype=np.float32)
        for bi in range(NB):
            for qi in range(NC):
                sidx = (ci * NB + bi) * NC + qi
                s0, s1 = seg_start[sidx], seg_start[sidx + 1]
                n = s1 - s0
                if n == 0:
                    continue
                o = seg_off[bi, qi]
                idx_pad[o:o + n] = lidx_s[s0:s1]
                dl_pad[o:o + n] = sl_s[s0:s1]
        a = idx_pad.reshape(CT, 8, 16)
        wrapped = a.transpose(2, 0, 1).reshape(16, CT * 8)
        idx16[ci] = np.tile(wrapped, (8, 1)).astype(np.int16)
        dl16[ci] = dl_pad.reshape(CT, 128).T.astype(ml_dtypes.bfloat16)
    return idx16, dl16, kq


def preprocess(cfg: Cfg, x, edge_index, W1, b1, W2, b2):
    import ml_dtypes
    N0, W, SHARD, NP = cfg.N0, cfg.W, cfg.SHARD, cfg.NP
    NB = cfg.NB
    x = np.asarray(x, dtype=np.float32)
    edge_index = np.asarray(edge_index)
    W1 = np.asarray(W1, np.float32)
    b1 = np.asarray(b1, np.float32)
    W2 = np.asarray(W2, np.float32)
    b2 = np.asarray(b2, np.float32)

    s = edge_index[0].astype(np.int64)
    d = edge_index[1].astype(np.int64)
    loops = np.arange(N0, dtype=np.int64)
    d_all = np.concatenate([d, loops])

    deg = np.bincount(d_all, minlength=NP).astype(np.float64)
    with np.errstate(divide="ignore"):
        dinv = np.where(deg > 0, 1.0 / np.sqrt(deg), 0.0).astype(np.float32)

    # degree-balanced serpentine block assignment over all W*NB blocks
    nblocks = W * NB
    order = np.argsort(-deg[:N0], kind="stable")
    all_ids = np.concatenate([order, np.arange(N0, NP, dtype=np.int64)])
    rr = np.arange(NP, dtype=np.int64)
    cyc = rr % (2 * nblocks)
    blk = np.where(cyc < nblocks, cyc, 2 * nblocks - 1 - cyc)
    slot_ctr = rr // (2 * nblocks) * 2 + (cyc >= nblocks).astype(np.int64)
    pos = (blk % W) * SHARD + (blk // W) * 128 + slot_ctr
    pos_of_node = np.empty(NP, dtype=np.int64)
    pos_of_node[all_ids] = pos
    node_of_pos = np.empty(NP, dtype=np.int64)
    node_of_pos[pos] = all_ids

    def gcls(p):
        """(class, lidx) of src pos: class = (bb//BH)*2 + c//4,
        lidx = (c%4)*RH + sl*BH + bb%BH."""
        BH, RH = cfg.BH, cfg.RH
        c = p // SHARD
        r = p % SHARD
        bb = r // 128
        sl = r % 128
        cls = (bb // BH) * 2 + c // 4
        lidx = (c % 4) * RH + sl * BH + (bb % BH)
        return cls, lidx

    # both layers exclude self-loops (handled on-device from residents)
    src_pos = pos_of_node[s]
    dst_pos = pos_of_node[d]
    cls, lidx = gcls(src_pos)
    idx16_1, dl1, kq1 = _route_edges(cfg, cls, lidx, dst_pos)
    idx16_2, dl2, kq2 = idx16_1, dl1, kq1

    dinv_pos = dinv[node_of_pos.reshape(W, SHARD)]  # [W, SHARD]

    xpad = np.zeros((NP, cfg.F), np.float32)
    xpad[:N0] = x
    ident = np.eye(128, dtype=np.float32)
    per_core = []
    for c in range(W):
        xs = xpad[node_of_pos[c * SHARD:(c + 1) * SHARD]]  # [SHARD, F] pos
        dpc = dinv_pos[c]
        inp = {
            "xT": np.ascontiguousarray(xs.T).astype(ml_dtypes.bfloat16),
            "w1": W1.astype(ml_dtypes.bfloat16),
            "b1col": b1.reshape(cfg.HID, 1).copy(),
            "w2p": np.pad(W2, ((0, 0), (0, cfg.CPAD - cfg.CLS))
                          ).astype(ml_dtypes.bfloat16),
            "b2rep": np.broadcast_to(
                np.pad(b2, (0, cfg.CPAD - cfg.CLS)), (128, cfg.CPAD)).copy(),
            "iota": np.broadcast_to(
                np.arange(128, dtype=np.float32),
                (128, 128)).astype(ml_dtypes.bfloat16),
            "ident": ident.astype(ml_dtypes.bfloat16),
            "idx1": idx16_1[c], "dl1": dl1[c],
            "dinv_pcT": np.ascontiguousarray(
                dpc.reshape(NB, 128).T).copy(),                 # [128, NB]
            "dinv_pr": np.broadcast_to(
                dpc, (128, SHARD)).astype(ml_dtypes.bfloat16),  # [128, SHARD]
        }
        per_core.append(inp)

    meta = Meta(kq1=kq1, kq2=kq2, node_of_pos=node_of_pos.reshape(W, SHARD))
    return per_core, meta, dinv


def postprocess(cfg: Cfg, outs, meta: Meta):
    """outs: list of [128, NB, CPAD] per core -> [N0, CLS] node order."""
    res = np.zeros((cfg.NP, cfg.CPAD), np.float32)
    for c in range(cfg.W):
        blockmaj = np.transpose(outs[c], (1, 0, 2)).reshape(
            cfg.SHARD, cfg.CPAD)
        res[meta.node_of_pos[c]] = blockmaj
    return res[:cfg.N0, :cfg.CLS]


def _superplan(cfg, kq):
    """Static per-super chunk layout: co[s], sct[s], nch[s][q], lbo[s][q][b]."""
    NS, SPB, NC = cfg.NS, cfg.SPB, cfg.NC
    co, sct, nch, lbo = [], [], [], []
    cursor = 0
    for s in range(NS):
        co.append(cursor)
        nq, lb = [], []
        loc = 0
        for qi in range(NC):
            lbq = []
            for j in range(SPB):
                lbq.append(loc)
                loc += int(kq[s * SPB + j, qi])
            lb.append(lbq)
            nq.append(sum(int(kq[s * SPB + j, qi]) for j in range(SPB)))
        nch.append(nq)
        lbo.append(lb)
        sct.append(loc)
        cursor += loc
    return co, sct, nch, lbo


def build(cfg: Cfg, meta: Meta):
    W, SHARD, NP, F = cfg.W, cfg.SHARD, cfg.NP, cfg.F
    HID, CPAD, NB, BH, RH, WS = (cfg.HID, cfg.CPAD, cfg.NB, cfg.BH, cfg.RH,
                                 cfg.WS)
    NC, SPB, NS, GT = cfg.NC, cfg.SPB, cfg.NS, cfg.GT
    kq1 = meta.kq1
    CT1 = int(kq1.sum())
    KT = F // 128
    GC = GT * 128  # cols per P1 load group
    NG = SHARD // GC
    NH = W * RH    # rows per half table
    HGRP = NS // 2  # supers per half (7)
    NH = W * RH    # rows per half table
    HGRP = NS // 2  # supers per half (7)

    co1, sct1, nch1, lbo1 = _superplan(cfg, kq1)
    MAXSCT = max(sct1)
    MAXNQ = max(max(r) for r in nch1)

    nc = bacc.Bacc("TRN2", target_bir_lowering=False, debug=False,
                   num_devices=W, num_swdge_queues=4)

    xT = nc.dram_tensor("xT", [F, SHARD], BF, kind="ExternalInput")
    w1 = nc.dram_tensor("w1", [F, HID], BF, kind="ExternalInput")
    b1col = nc.dram_tensor("b1col", [HID, 1], FP, kind="ExternalInput")
    w2p = nc.dram_tensor("w2p", [HID, CPAD], BF, kind="ExternalInput")
    b2rep = nc.dram_tensor("b2rep", [128, CPAD], FP, kind="ExternalInput")
    iota = nc.dram_tensor("iota", [128, 128], BF, kind="ExternalInput")
    identt = nc.dram_tensor("ident", [128, 128], BF, kind="ExternalInput")
    idx1 = nc.dram_tensor("idx1", [128, CT1 * 8], mybir.dt.int16,
                          kind="ExternalInput")
    dl1 = nc.dram_tensor("dl1", [128, CT1], BF, kind="ExternalInput")
    dinv_pcT = nc.dram_tensor("dinv_pcT", [128, NB], FP, kind="ExternalInput")
    dinv_pr = nc.dram_tensor("dinv_pr", [128, SHARD], BF, kind="ExternalInput")
    out_s = nc.dram_tensor("out_s", [128, NB, CPAD], FP, kind="ExternalOutput")

    ag1_inA = nc.dram_tensor("ag1_inA", [128, BH, HID], BF)
    ag1_inB = nc.dram_tensor("ag1_inB", [128, BH, HID], BF)
    ag1_outA = nc.dram_tensor("ag1_outA", [NH, HID], BF, addr_space="Shared")
    ag1_outB = nc.dram_tensor("ag1_outB", [NH, HID], BF, addr_space="Shared")
    ag2_inA = nc.dram_tensor("ag2_inA", [128, BH, CPAD], BF)
    ag2_inB = nc.dram_tensor("ag2_inB", [128, BH, CPAD], BF)
    ag2_outA = nc.dram_tensor("ag2_outA", [NH, CPAD], BF, addr_space="Shared")
    ag2_outB = nc.dram_tensor("ag2_outB", [NH, CPAD], BF, addr_space="Shared")

    def ag(ins_ap, outs_ap):
        nc.gpsimd.collective_compute(
            "AllGather", mybir.AluOpType.bypass,
            replica_groups=[list(range(W))],
            ins=[ins_ap], outs=[outs_ap],
        )

    with tile.TileContext(nc) as tc:
        with (
            tc.tile_pool(name="const", bufs=1) as cpool,
            tc.tile_pool(name="p1x", bufs=2) as p1pool,
            tc.tile_pool(name="meta1", bufs=2) as mpool,
            tc.tile_pool(name="gath", bufs=2) as gpool,
            tc.tile_pool(name="indp", bufs=2) as ipool,
            tc.tile_pool(name="mid", bufs=3) as midpool,
            tc.tile_pool(name="outg", bufs=2) as opool,
            tc.tile_pool(name="ps", bufs=1, space="PSUM") as pspool,
            tc.tile_pool(name="psw", bufs=1, space="PSUM") as pswpool,
        ):
            # ---- constants ----
            iota_t = cpool.tile([128, 128], BF)
            nc.sync.dma_start(out=iota_t[:, :], in_=iota[:, :])
            ident_t = cpool.tile([128, 128], BF)
            nc.sync.dma_start(out=ident_t[:, :], in_=identt[:, :])
            b1_t = cpool.tile([HID, 1], FP)
            nc.sync.dma_start(out=b1_t[:, :], in_=b1col[:, :])
            w2_t = cpool.tile([HID, CPAD], BF)
            nc.sync.dma_start(out=w2_t[:, :], in_=w2p[:, :])
            b2_t = cpool.tile([128, CPAD], FP)
            nc.sync.dma_start(out=b2_t[:, :], in_=b2rep[:, :])
            w1k_t = cpool.tile([128, KT, HID], BF)
            for k in range(KT):
                nc.sync.dma_start(out=w1k_t[:, k, :],
                                  in_=w1[k * 128:(k + 1) * 128, :])
            dpcT_t = cpool.tile([128, NB], FP)
            nc.sync.dma_start(out=dpcT_t[:, :], in_=dinv_pcT[:, :])
            dpr_t = cpool.tile([128, SHARD], BF)
            nc.sync.dma_start(out=dpr_t[:, :], in_=dinv_pr[:, :])
            h1p_res = cpool.tile([128, NB, HID], BF)
            h2p_res = cpool.tile([128, NB, CPAD], BF)

            qctr = [0]

            def next_q():
                qctr[0] = (qctr[0] + 1) % 4
                return qctr[0]

            # ---- P1: h1' = dinv .* (x @ W1), streamed col groups ----
            for g in range(NG):
                xt = p1pool.tile([128, KT, GC], BF, tag="xt")
                for k in range(KT):
                    nc.sync.dma_start(
                        out=xt[:, k, :],
                        in_=xT[k * 128:(k + 1) * 128, g * GC:(g + 1) * GC])
                for t in range(GT):
                    blk = g * GT + t
                    psh = pspool.tile([128, HID], FP, space="PSUM",
                                      tag=f"acc{t}", name=f"acc{t}")
                    for k in range(KT):
                        nc.tensor.matmul(
                            out=psh[:, :],
                            lhsT=xt[:, k, t * 128:(t + 1) * 128],
                            rhs=w1k_t[:, k, :],
                            start=(k == 0), stop=(k == KT - 1))
                    nc.scalar.activation(
                        out=h1p_res[:, blk, :], in_=psh[:, :],
                        func=mybir.ActivationFunctionType.Copy,
                        scale=dpcT_t[:, blk:blk + 1])
                if g < HGRP:
                    nc.sync.dma_start(
                        out=ag1_inA[:, g * GT:(g + 1) * GT, :],
                        in_=h1p_res[:, g * GT:(g + 1) * GT, :])
                else:
                    nc.sync.dma_start(
                        out=ag1_inB[:, (g - HGRP) * GT:(g - HGRP + 1) * GT, :],
                        in_=h1p_res[:, g * GT:(g + 1) * GT, :])
                if g == HGRP - 1:
                    ag(ag1_inA[:, :, :], ag1_outA[:, :])
            ag(ag1_inB[:, :, :], ag1_outB[:, :])

            # ---- P3: L1 aggregation + relu + @W2 -> h2' ----
            for s in range(NS):
                sct = sct1[s]
                o0 = co1[s]
                ixt = mpool.tile([128, MAXSCT * 8], mybir.dt.int16, tag="ix")
                nc.sync.dma_start(out=ixt[:, :sct * 8],
                                  in_=idx1[:, o0 * 8:(o0 + sct) * 8])
                dlt = mpool.tile([128, MAXSCT], BF, tag="dl")
                nc.sync.dma_start(out=dlt[:, :sct], in_=dl1[:, o0:o0 + sct])

                gbuf = gpool.tile([128, MAXSCT, HID], BF, tag="g")
                lq = 0
                for qi in range(NC):
                    nch = nch1[s][qi]
                    tbl = ag1_outA if qi < 2 else ag1_outB
                    nc.gpsimd.dma_gather(
                        gbuf[:, lq:lq + nch, :],
                        tbl[(qi % 2) * WS:(qi % 2 + 1) * WS, :],
                        ixt[:, lq * 8:(lq + nch) * 8],
                        nch * 128, nch * 128, HID,
                        single_packet=False, queue_num=next_q(),
                    )
                    lq += nch

                accs = [pspool.tile([128, 128], FP, space="PSUM",
                                    tag=f"acc{j}", name=f"acc{j}")
                        for j in range(SPB)]
                # self-loop: ps1 := h1p_blk^T via identity matmul
                for j in range(SPB):
                    bb = s * SPB + j
                    nc.tensor.matmul(out=accs[j][:, :],
                                     lhsT=h1p_res[:, bb, :],
                                     rhs=ident_t[:, :],
                                     start=True, stop=False)
                for qi in range(NC):
                    nch = nch1[s][qi]
                    lq = lbo1[s][qi][0]
                    ind = ipool.tile([128, MAXNQ, 128], BF, tag="i")
                    nc.vector.tensor_tensor(
                        out=ind[:, :nch, :],
                        in0=dlt[:, lq:lq + nch].to_broadcast([128, nch, 128]),
                        in1=iota_t[:, None, :].to_broadcast([128, nch, 128]),
                        op=mybir.AluOpType.is_equal,
                    )
                    for j in range(SPB):
                        bb = s * SPB + j
                        kq = int(kq1[bb, qi])
                        ps1 = accs[j]
                        boff = lbo1[s][qi][j]
                        for k in range(kq):
                            ck = boff + k
                            nc.tensor.matmul(
                                out=ps1[:, :],
                                lhsT=gbuf[:, ck, :],
                                rhs=ind[:, ck - lq, :],
                                start=False,
                                stop=(qi == NC - 1 and k == kq - 1))

                for j in range(SPB):
                    bb = s * SPB + j
                    ps1 = accs[j]
                    t1 = midpool.tile([128, 128], FP, tag="t1")
                    nc.vector.tensor_tensor(
                        out=t1[:, :], in0=ps1[:, :],
                        in1=dpr_t[:, bb * 128:(bb + 1) * 128],
                        op=mybir.AluOpType.mult)
                    r1 = midpool.tile([128, 128], BF, tag="r1")
                    nc.scalar.activation(
                        out=r1[:, :], in_=t1[:, :],
                        func=mybir.ActivationFunctionType.Relu,
                        bias=b1_t[:, :1])
                    ps2 = pswpool.tile([128, CPAD], FP, space="PSUM",
                                       tag="accw")
                    nc.tensor.matmul(out=ps2[:, :], lhsT=r1[:, :],
                                     rhs=w2_t[:, :], start=True, stop=True)
                    nc.scalar.activation(
                        out=h2p_res[:, bb, :], in_=ps2[:, :],
                        func=mybir.ActivationFunctionType.Copy,
                        scale=dpcT_t[:, bb:bb + 1])
                if s < HGRP:
                    nc.sync.dma_start(
                        out=ag2_inA[:, s * SPB:(s + 1) * SPB, :],
                        in_=h2p_res[:, s * SPB:(s + 1) * SPB, :])
                else:
                    sb = s - HGRP
                    nc.sync.dma_start(
                        out=ag2_inB[:, sb * SPB:(sb + 1) * SPB, :],
                        in_=h2p_res[:, s * SPB:(s + 1) * SPB, :])
                if s == HGRP - 1:
                    ag(ag2_inA[:, :, :], ag2_outA[:, :])
            ag(ag2_inB[:, :, :], ag2_outB[:, :])

            # ---- P5: L2 aggregation + self-loop + b2 -> out ----
            for s in range(NS):
                sct = sct1[s]
                o0 = co1[s]
                ixt = mpool.tile([128, MAXSCT * 8], mybir.dt.int16, tag="ix")
                nc.sync.dma_start(out=ixt[:, :sct * 8],
                                  in_=idx1[:, o0 * 8:(o0 + sct) * 8])
                dlt = mpool.tile([128, MAXSCT], BF, tag="dl")
                nc.sync.dma_start(out=dlt[:, :sct], in_=dl1[:, o0:o0 + sct])

                gbuf = gpool.tile([128, MAXSCT, CPAD], BF, tag="g")
                lq = 0
                for qi in range(NC):
                    nch = nch1[s][qi]
                    tbl = ag2_outA if qi < 2 else ag2_outB
                    nc.gpsimd.dma_gather(
                        gbuf[:, lq:lq + nch, :],
                        tbl[(qi % 2) * WS:(qi % 2 + 1) * WS, :],
                        ixt[:, lq * 8:(lq + nch) * 8],
                        nch * 128, nch * 128, CPAD,
                        single_packet=False, queue_num=next_q(),
                    )
                    lq += nch

                accs = [pspool.tile([128, CPAD], FP, space="PSUM",
                                    tag=f"acc{j}", name=f"acc{j}")
                        for j in range(SPB)]
                for qi in range(NC):
                    nch = nch1[s][qi]
                    lq = lbo1[s][qi][0]
                    ind = ipool.tile([128, MAXNQ, 128], BF, tag="i")
                    nc.vector.tensor_tensor(
                        out=ind[:, :nch, :],
                        in0=dlt[:, lq:lq + nch].to_broadcast([128, nch, 128]),
                        in1=iota_t[:, None, :].to_broadcast([128, nch, 128]),
                        op=mybir.AluOpType.is_equal,
                    )
                    for j in range(SPB):
                        bb = s * SPB + j
                        kq = int(kq1[bb, qi])
                        ps3 = accs[j]
                        boff = lbo1[s][qi][j]
                        for k in range(kq):
                            ck = boff + k
                            nc.tensor.matmul(
                                out=ps3[:, :],
                                lhsT=ind[:, ck - lq, :],
                                rhs=gbuf[:, ck, :],
                                start=(qi == 0 and k == 0),
                                stop=(qi == NC - 1 and k == kq - 1))

                og = opool.tile([128, SPB, CPAD], FP, tag="og")
                for j in range(SPB):
                    bb = s * SPB + j
                    ps3 = accs[j]
                    u = midpool.tile([128, CPAD], FP, tag="u")
                    nc.vector.tensor_tensor(
                        out=u[:, :], in0=ps3[:, :], in1=h2p_res[:, bb, :],
                        op=mybir.AluOpType.add)
                    v = midpool.tile([128, CPAD], FP, tag="v")
                    nc.scalar.activation(
                        out=v[:, :], in_=u[:, :],
                        func=mybir.ActivationFunctionType.Copy,
                        scale=dpcT_t[:, bb:bb + 1])
                    nc.vector.tensor_tensor(
                        out=og[:, j, :], in0=v[:, :], in1=b2_t[:, :],
                        op=mybir.AluOpType.add)
                nc.sync.dma_start(out=out_s[:, s * SPB:(s + 1) * SPB, :],
                                  in_=og[:, :, :])

    nc.compile()
    return nc


# ======================================================================
# kernel() entry point
# ======================================================================
import os as _os


LAST_EXEC_NS = None
LAST_RES = None


def kernel(x, edge_index, W1, b1, W2, b2):
    """Full-input GCN kernel: shards across 8 NeuronCores internally."""
    global LAST_EXEC_NS, LAST_RES
    import numpy as _np

    trace = bool(int(_os.environ.get("GCN_TRACE", "0")))
    if trace:
        try:
            import sys as _sys
            import types as _types
            from trn_agent_boot.trn_boot import _ntff_profile_via_ctypes
            if "antenv.axon_hooks" not in _sys.modules:
                _hook = _ntff_profile_via_ctypes("/opt/axon/libaxon_pjrt.so")
                _m = _types.ModuleType("antenv.axon_hooks")
                _m.get_axon_ntff_profile_hook = lambda: _hook
                _m.set_axon_ntff_profile_hook = lambda h: None
                _sys.modules["antenv.axon_hooks"] = _m
        except Exception:
            trace = False

    from concourse.bass_utils import run_bass_kernel_spmd

    cfg = Cfg()
    per_core, meta, _ = preprocess(cfg, x, edge_index, W1, b1, W2, b2)
    nc = build(cfg, meta)
    res = run_bass_kernel_spmd(
        nc, per_core, core_ids=list(range(cfg.W)), trace=trace,
    )
    LAST_EXEC_NS = res.exec_time_ns
    LAST_RES = res
    outs = [res.results[c]["out_s"] for c in range(cfg.W)]
    return _np.ascontiguousarray(postprocess(cfg, outs, meta).astype(_np.float32))


# revision 14
# speedup vs baseline: 1.2353x; 1.0080x over previous
"""GCN 2-layer kernel for trn2: host preprocessing + Bass kernel builder.

Math (per GCNConv, PyG-style):
  out = D^-1/2 (A+I) D^-1/2 (X W) + b
Layer1 -> relu -> Layer2.

Device plan (8 cores, SPMD), all tensors in pos-order with p-major DRAM
layout [128, NB, feat] (row of node (bb, sl) lives at [sl, bb, :]).
AllGathers are split into two half-collectives (blocks 0-48 / 49-97) so
each half overlaps compute; gather tables are the two half-outputs, each
addressed through 2 int16 windows (cores 0-3 / 4-7).

  P1: h1' = dinv .* (x_pos @ W1) -> resident h1p + ag1_inA/B stores
  AG1a (after group 6), AG1b (at end of P1)
  P3: per super (7 dst blocks): 4 dma_gather (one per (half, window)),
      is_equal indicator per (super, class), chunk matmuls accumulate in
      7 psum banks; self-loop via identity matmul from resident h1p;
      then per block: dinv scale -> relu+b1 -> @W2 -> dinv scale -> h2'
      (resident + stores to ag2_inA/B)
  AG2a (after super 6), AG2b (end of P3)
  P5: same gathers; chunk matmuls [dst, CPAD]; self-loop via resident
      h2' VE-add; dinv scale + b2 -> batched store to out_s
Host: unpermute rows, slice [:N0, :CLS].
"""

from dataclasses import dataclass

import numpy as np

import concourse.bass as bass
import concourse.mybir as mybir
import concourse.tile as tile
from concourse import bacc

FP = mybir.dt.float32
BF = mybir.dt.bfloat16


@dataclass
class Cfg:
    N0: int = 100000     # real nodes
    W: int = 8           # cores
    SHARD: int = 12544   # nodes per core (98 * 128)
    F: int = 512         # in features
    HID: int = 128
    CLS: int = 40
    CPAD: int = 128
    NC: int = 4          # gather classes (half, core-window)
    SPB: int = 7         # dst blocks per super
    GT: int = 7          # P1 col-tiles per load group

    @property
    def NP(self):
        return self.W * self.SHARD

    @property
    def NB(self):
        return self.SHARD // 128

    @property
    def BH(self):
        return self.NB // 2  # blocks per half (49)

    @property
    def RH(self):
        return 128 * self.BH  # rows per core per half table (6272)

    @property
    def WS(self):
        return 4 * self.RH   # rows per int16 window (25088)

    @property
    def NS(self):
        return self.NB // self.SPB


@dataclass
class Meta:
    kq1: np.ndarray = None   # [NB, NC] chunks per (block, class)
    kq2: np.ndarray = None
    node_of_pos: np.ndarray = None  # [W, SHARD] -> node id


def _route_edges(cfg, cls, lidx, dst_pos):
    """Bucket edges by (core of dst, super, class of src, block).
    Chunk stream order per core: (super, class, bb in super, chunk).
    Returns idx16 [W,128,CT*8], dl bf16 [W,128,CT], kq [NB,NC]."""
    import ml_dtypes
    W, SHARD, NC = cfg.W, cfg.SHARD, cfg.NC
    NB, SPB, NS = cfg.NB, cfg.SPB, cfg.NS

    c = dst_pos // SHARD
    r = dst_pos % SHARD
    bb = r // 128
    sl = r % 128

    key = (c * NB + bb) * NC + cls
    order = np.argsort(key, kind="stable")
    lidx_s = lidx[order]
    sl_s = sl[order]

    nseg = W * NB * NC
    counts = np.bincount(key[order], minlength=nseg).reshape(W, NB, NC)
    kq = np.maximum(
        np.ceil(counts.max(axis=0) / 128).astype(np.int64), 1)  # [NB, NC]

    # stream offset of (bb, class) segment: order (s, class, bb in s)
    seg_off = np.zeros((NB, NC), dtype=np.int64)
    cursor = 0
    for s in range(NS):
        for qi in range(NC):
            for j in range(SPB):
                b = s * SPB + j
                seg_off[b, qi] = cursor
                cursor += kq[b, qi] * 128
    CT = int(kq.sum())
    EPAD = CT * 128
    assert cursor == EPAD

    seg_start = np.zeros(nseg + 1, dtype=np.int64)
    seg_start[1:] = np.cumsum(counts.reshape(-1))

    idx16 = np.zeros((W, 128, CT * 8), dtype=np.int16)
    dl16 = np.zeros((W, 128, CT), dtype=ml_dtypes.bfloat16)
    for ci in range(W):
        idx_pad = np.zeros(EPAD, dtype=np.int64)
        dl_pad = np.full(EPAD, -1.0, dtype=np.float32)
        for bi in range(NB):
            for qi in range(NC):
                sidx = (ci * NB + bi) * NC + qi
                s0, s1 = seg_start[sidx], seg_start[sidx + 1]
                n = s1 - s0
                if n == 0:
                    continue
                o = seg_off[bi, qi]
                idx_pad[o:o + n] = lidx_s[s0:s1]
                dl_pad[o:o + n] = sl_s[s0:s1]
        a = idx_pad.reshape(CT, 8, 16)
        wrapped = a.transpose(2, 0, 1).reshape(16, CT * 8)
        idx16[ci] = np.tile(wrapped, (8, 1)).astype(np.int16)
        dl16[ci] = dl_pad.reshape(CT, 128).T.astype(ml_dtypes.bfloat16)
    return idx16, dl16, kq


def preprocess(cfg: Cfg, x, edge_index, W1, b1, W2, b2):
    import ml_dtypes
    N0, W, SHARD, NP = cfg.N0, cfg.W, cfg.SHARD, cfg.NP
    NB = cfg.NB
    x = np.asarray(x, dtype=np.float32)
    edge_index = np.asarray(edge_index)
    W1 = np.asarray(W1, np.float32)
    b1 = np.asarray(b1, np.float32)
    W2 = np.asarray(W2, np.float32)
    b2 = np.asarray(b2, np.float32)

    s = edge_index[0].astype(np.int64)
    d = edge_index[1].astype(np.int64)
    loops = np.arange(N0, dtype=np.int64)
    d_all = np.concatenate([d, loops])

    deg = np.bincount(d_all, minlength=NP).astype(np.float64)
    with np.errstate(divide="ignore"):
        dinv = np.where(deg > 0, 1.0 / np.sqrt(deg), 0.0).astype(np.float32)

    # degree-balanced serpentine block assignment over all W*NB blocks
    nblocks = W * NB
    order = np.argsort(-deg[:N0], kind="stable")
    all_ids = np.concatenate([order, np.arange(N0, NP, dtype=np.int64)])
    rr = np.arange(NP, dtype=np.int64)
    cyc = rr % (2 * nblocks)
    blk = np.where(cyc < nblocks, cyc, 2 * nblocks - 1 - cyc)
    slot_ctr = rr // (2 * nblocks) * 2 + (cyc >= nblocks).astype(np.int64)
    pos = (blk % W) * SHARD + (blk // W) * 128 + slot_ctr
    pos_of_node = np.empty(NP, dtype=np.int64)
    pos_of_node[all_ids] = pos
    node_of_pos = np.empty(NP, dtype=np.int64)
    node_of_pos[pos] = all_ids

    def gcls(p):
        """(class, lidx) of src pos: class = (bb//BH)*2 + c//4,
        lidx = (c%4)*RH + sl*BH + bb%BH."""
        BH, RH = cfg.BH, cfg.RH
        c = p // SHARD
        r = p % SHARD
        bb = r // 128
        sl = r % 128
        cls = (bb // BH) * 2 + c // 4
        lidx = (c % 4) * RH + sl * BH + (bb % BH)
        return cls, lidx

    # both layers exclude self-loops (handled on-device from residents)
    src_pos = pos_of_node[s]
    dst_pos = pos_of_node[d]
    cls, lidx = gcls(src_pos)
    idx16_1, dl1, kq1 = _route_edges(cfg, cls, lidx, dst_pos)
    idx16_2, dl2, kq2 = idx16_1, dl1, kq1

    dinv_pos = dinv[node_of_pos.reshape(W, SHARD)]  # [W, SHARD]

    xpad = np.zeros((NP, cfg.F), np.float32)
    xpad[:N0] = x
    ident = np.eye(128, dtype=np.float32)
    per_core = []
    for c in range(W):
        xs = xpad[node_of_pos[c * SHARD:(c + 1) * SHARD]]  # [SHARD, F] pos
        dpc = dinv_pos[c]
        inp = {
            "xT": np.ascontiguousarray(xs.T).astype(ml_dtypes.bfloat16),
            "w1": W1.astype(ml_dtypes.bfloat16),
            "b1col": b1.reshape(cfg.HID, 1).copy(),
            "w2p": np.pad(W2, ((0, 0), (0, cfg.CPAD - cfg.CLS))
                          ).astype(ml_dtypes.bfloat16),
            "b2rep": np.broadcast_to(
                np.pad(b2, (0, cfg.CPAD - cfg.CLS)), (128, cfg.CPAD)).copy(),
            "iota": np.broadcast_to(
                np.arange(128, dtype=np.float32),
                (128, 128)).astype(ml_dtypes.bfloat16),
            "ident": ident.astype(ml_dtypes.bfloat16),
            "idx1": idx16_1[c], "dl1": dl1[c],
            "dinv_pcT": np.ascontiguousarray(
                dpc.reshape(NB, 128).T).copy(),                 # [128, NB]
            "dinv_pr": np.broadcast_to(
                dpc, (128, SHARD)).astype(ml_dtypes.bfloat16),  # [128, SHARD]
        }
        per_core.append(inp)

    meta = Meta(kq1=kq1, kq2=kq2, node_of_pos=node_of_pos.reshape(W, SHARD))
    return per_core, meta, dinv


def postprocess(cfg: Cfg, outs, meta: Meta):
    """outs: list of [128, NB, CPAD] per core -> [N0, CLS] node order."""
    res = np.zeros((cfg.NP, cfg.CPAD), np.float32)
    for c in range(cfg.W):
        blockmaj = np.transpose(outs[c], (1, 0, 2)).reshape(
            cfg.SHARD, cfg.CPAD)
        res[meta.node_of_pos[c]] = blockmaj
    return res[:cfg.N0, :cfg.CLS]


def _superplan(cfg, kq):
    """Static per-super chunk layout: co[s], sct[s], nch[s][q], lbo[s][q][b]."""
    NS, SPB, NC = cfg.NS, cfg.SPB, cfg.NC
    co, sct, nch, lbo = [], [], [], []
    cursor = 0
    for s in range(NS):
        co.append(cursor)
        nq, lb = [], []
        loc = 0
        for qi in range(NC):
            lbq = []
            for j in range(SPB):
                lbq.append(loc)
                loc += int(kq[s * SPB + j, qi])
            lb.append(lbq)
            nq.append(sum(int(kq[s * SPB + j, qi]) for j in range(SPB)))
        nch.append(nq)
        lbo.append(lb)
        sct.append(loc)
        cursor += loc
    return co, sct, nch, lbo


def build(cfg: Cfg, meta: Meta):
    W, SHARD, NP, F = cfg.W, cfg.SHARD, cfg.NP, cfg.F
    HID, CPAD, NB, BH, RH, WS = (cfg.HID, cfg.CPAD, cfg.NB, cfg.BH, cfg.RH,
                                 cfg.WS)
    NC, SPB, NS, GT = cfg.NC, cfg.SPB, cfg.NS, cfg.GT
    kq1 = meta.kq1
    CT1 = int(kq1.sum())
    KT = F // 128
    GC = GT * 128  # cols per P1 load group
    NG = SHARD // GC
    NH = W * RH    # rows per half table
    HGRP = NS // 2  # supers per half (7)
    NH = W * RH    # rows per half table
    HGRP = NS // 2  # supers per half (7)

    co1, sct1, nch1, lbo1 = _superplan(cfg, kq1)
    MAXSCT = max(sct1)
    MAXNQ = max(max(r) for r in nch1)

    nc = bacc.Bacc("TRN2", target_bir_lowering=False, debug=False,
                   num_devices=W, num_swdge_queues=4)

    xT = nc.dram_tensor("xT", [F, SHARD], BF, kind="ExternalInput")
    w1 = nc.dram_tensor("w1", [F, HID], BF, kind="ExternalInput")
    b1col = nc.dram_tensor("b1col", [HID, 1], FP, kind="ExternalInput")
    w2p = nc.dram_tensor("w2p", [HID, CPAD], BF, kind="ExternalInput")
    b2rep = nc.dram_tensor("b2rep", [128, CPAD], FP, kind="ExternalInput")
    iota = nc.dram_tensor("iota", [128, 128], BF, kind="ExternalInput")
    identt = nc.dram_tensor("ident", [128, 128], BF, kind="ExternalInput")
    idx1 = nc.dram_tensor("idx1", [128, CT1 * 8], mybir.dt.int16,
                          kind="ExternalInput")
    dl1 = nc.dram_tensor("dl1", [128, CT1], BF, kind="ExternalInput")
    dinv_pcT = nc.dram_tensor("dinv_pcT", [128, NB], FP, kind="ExternalInput")
    dinv_pr = nc.dram_tensor("dinv_pr", [128, SHARD], BF, kind="ExternalInput")
    out_s = nc.dram_tensor("out_s", [128, NB, CPAD], FP, kind="ExternalOutput")

    ag1_inA = nc.dram_tensor("ag1_inA", [128, BH, HID], BF)
    ag1_inB = nc.dram_tensor("ag1_inB", [128, BH, HID], BF)
    ag1_outA = nc.dram_tensor("ag1_outA", [NH, HID], BF, addr_space="Shared")
    ag1_outB = nc.dram_tensor("ag1_outB", [NH, HID], BF, addr_space="Shared")
    ag2_inA = nc.dram_tensor("ag2_inA", [128, BH, CPAD], BF)
    ag2_inB = nc.dram_tensor("ag2_inB", [128, BH, CPAD], BF)
    ag2_outA = nc.dram_tensor("ag2_outA", [NH, CPAD], BF, addr_space="Shared")
    ag2_outB = nc.dram_tensor("ag2_outB", [NH, CPAD], BF, addr_space="Shared")

    def ag(ins_ap, outs_ap):
        nc.gpsimd.collective_compute(
            "AllGather", mybir.AluOpType.bypass,
            replica_groups=[list(range(W))],
            ins=[ins_ap], outs=[outs_ap],
        )

    with tile.TileContext(nc) as tc:
        with (
            tc.tile_pool(name="const", bufs=1) as cpool,
            tc.tile_pool(name="p1x", bufs=2) as p1pool,
            tc.tile_pool(name="meta1", bufs=2) as mpool,
            tc.tile_pool(name="gath", bufs=2) as gpool,
            tc.tile_pool(name="indp", bufs=2) as ipool,
            tc.tile_pool(name="mid", bufs=3) as midpool,
            tc.tile_pool(name="outg", bufs=2) as opool,
            tc.tile_pool(name="ps", bufs=1, space="PSUM") as pspool,
            tc.tile_pool(name="psw", bufs=1, space="PSUM") as pswpool,
        ):
            # ---- constants ----
            iota_t = cpool.tile([128, 128], BF)
            nc.sync.dma_start(out=iota_t[:, :], in_=iota[:, :])
            ident_t = cpool.tile([128, 128], BF)
            nc.sync.dma_start(out=ident_t[:, :], in_=identt[:, :])
            b1_t = cpool.tile([HID, 1], FP)
            nc.sync.dma_start(out=b1_t[:, :], in_=b1col[:, :])
            w2_t = cpool.tile([HID, CPAD], BF)
            nc.sync.dma_start(out=w2_t[:, :], in_=w2p[:, :])
            b2_t = cpool.tile([128, CPAD], FP)
            nc.sync.dma_start(out=b2_t[:, :], in_=b2rep[:, :])
            w1k_t = cpool.tile([128, KT, HID], BF)
            for k in range(KT):
                nc.sync.dma_start(out=w1k_t[:, k, :],
                                  in_=w1[k * 128:(k + 1) * 128, :])
            dpcT_t = cpool.tile([128, NB], FP)
            nc.sync.dma_start(out=dpcT_t[:, :], in_=dinv_pcT[:, :])
            dpr_t = cpool.tile([128, SHARD], BF)
            nc.sync.dma_start(out=dpr_t[:, :], in_=dinv_pr[:, :])
            h1p_res = cpool.tile([128, NB, HID], BF)
            h2p_res = cpool.tile([128, NB, CPAD], BF)

            qctr = [0]

            def next_q():
                qctr[0] = (qctr[0] + 1) % 4
                return qctr[0]

            # ---- P1: h1' = dinv .* (x @ W1), streamed col groups ----
            for g in range(NG):
                xt = p1pool.tile([128, KT, GC], BF, tag="xt")
                for k in range(KT):
                    nc.sync.dma_start(
                        out=xt[:, k, :],
                        in_=xT[k * 128:(k + 1) * 128, g * GC:(g + 1) * GC])
                for t in range(GT):
                    blk = g * GT + t
                    psh = pspool.tile([128, HID], FP, space="PSUM",
                                      tag=f"acc{t}", name=f"acc{t}")
                    for k in range(KT):
                        nc.tensor.matmul(
                            out=psh[:, :],
                            lhsT=xt[:, k, t * 128:(t + 1) * 128],
                            rhs=w1k_t[:, k, :],
                            start=(k == 0), stop=(k == KT - 1))
                    nc.scalar.activation(
                        out=h1p_res[:, blk, :], in_=psh[:, :],
                        func=mybir.ActivationFunctionType.Copy,
                        scale=dpcT_t[:, blk:blk + 1])
                if g < HGRP:
                    nc.sync.dma_start(
                        out=ag1_inA[:, g * GT:(g + 1) * GT, :],
                        in_=h1p_res[:, g * GT:(g + 1) * GT, :])
                else:
                    nc.sync.dma_start(
                        out=ag1_inB[:, (g - HGRP) * GT:(g - HGRP + 1) * GT, :],
                        in_=h1p_res[:, g * GT:(g + 1) * GT, :])
                if g == HGRP - 1:
                    ag(ag1_inA[:, :, :], ag1_outA[:, :])
            ag(ag1_inB[:, :, :], ag1_outB[:, :])

            # ---- P3: L1 aggregation + relu + @W2 -> h2' ----
            for s in range(NS):
                sct = sct1[s]
                o0 = co1[s]
                ixt = mpool.tile([128, MAXSCT * 8], mybir.dt.int16, tag="ix")
                nc.sync.dma_start(out=ixt[:, :sct * 8],
                                  in_=idx1[:, o0 * 8:(o0 + sct) * 8])
                dlt = mpool.tile([128, MAXSCT], BF, tag="dl")
                nc.sync.dma_start(out=dlt[:, :sct], in_=dl1[:, o0:o0 + sct])

                gbuf = gpool.tile([128, MAXSCT, HID], BF, tag="g")
                lq = 0
                for qi in range(NC):
                    nch = nch1[s][qi]
                    tbl = ag1_outA if qi < 2 else ag1_outB
                    nc.gpsimd.dma_gather(
                        gbuf[:, lq:lq + nch, :],
                        tbl[(qi % 2) * WS:(qi % 2 + 1) * WS, :],
                        ixt[:, lq * 8:(lq + nch) * 8],
                        nch * 128, nch * 128, HID,
                        single_packet=False, queue_num=next_q(),
                    )
                    lq += nch

                accs = [pspool.tile([128, 128], FP, space="PSUM",
                                    tag=f"acc{j}", name=f"acc{j}")
                        for j in range(SPB)]
                # self-loop: ps1 := h1p_blk^T via identity matmul
                for j in range(SPB):
                    bb = s * SPB + j
                    nc.tensor.matmul(out=accs[j][:, :],
                                     lhsT=h1p_res[:, bb, :],
                                     rhs=ident_t[:, :],
                                     start=True, stop=False)
                for qi in range(NC):
                    nch = nch1[s][qi]
                    lq = lbo1[s][qi][0]
                    ind = ipool.tile([128, MAXNQ, 128], BF, tag="i")
                    nc.vector.tensor_tensor(
                        out=ind[:, :nch, :],
                        in0=dlt[:, lq:lq + nch].to_broadcast([128, nch, 128]),
                        in1=iota_t[:, None, :].to_broadcast([128, nch, 128]),
                        op=mybir.AluOpType.is_equal,
                    )
                    for j in range(SPB):
                        bb = s * SPB + j
                        kq = int(kq1[bb, qi])
                        ps1 = accs[j]
                        boff = lbo1[s][qi][j]
                        for k in range(kq):
                            ck = boff + k
                            nc.tensor.matmul(
                                out=ps1[:, :],
                                lhsT=gbuf[:, ck, :],
                                rhs=ind[:, ck - lq, :],
                                start=False,
                                stop=(qi == NC - 1 and k == kq - 1))

                for j in range(SPB):
                    bb = s * SPB + j
                    ps1 = accs[j]
                    t1 = midpool.tile([128, 128], FP, tag="t1")
                    nc.vector.tensor_tensor(
                        out=t1[:, :], in0=ps1[:, :],
                        in1=dpr_t[:, bb * 128:(bb + 1) * 128],
                        op=mybir.AluOpType.mult)
                    r1 = midpool.tile([128, 128], BF, tag="r1")
                    nc.scalar.activation(
                        out=r1[:, :], in_=t1[:, :],
                        func=mybir.ActivationFunctionType.Relu,
                        bias=b1_t[:, :1])
                    ps2 = pswpool.tile([128, CPAD], FP, space="PSUM",
                                       tag="accw")
                    nc.tensor.matmul(out=ps2[:, :], lhsT=r1[:, :],
                                     rhs=w2_t[:, :], start=True, stop=True)
                    nc.scalar.activation(
                        out=h2p_res[:, bb, :], in_=ps2[:, :],
                        func=mybir.ActivationFunctionType.Copy,
                        scale=dpcT_t[:, bb:bb + 1])
                if s < HGRP:
                    nc.sync.dma_start(
                        out=ag2_inA[:, s * SPB:(s + 1) * SPB, :],
                        in_=h2p_res[:, s * SPB:(s + 1) * SPB, :])
                else:
                    sb = s - HGRP
                    nc.sync.dma_start(
                        out=ag2_inB[:, sb * SPB:(sb + 1) * SPB, :],
                        in_=h2p_res[:, s * SPB:(s + 1) * SPB, :])
                if s == HGRP - 1:
                    ag(ag2_inA[:, :, :], ag2_outA[:, :])
            ag(ag2_inB[:, :, :], ag2_outB[:, :])

            # ---- P5: L2 aggregation + self-loop + b2 -> out ----
            for s in range(NS):
                sct = sct1[s]
                o0 = co1[s]
                ixt = mpool.tile([128, MAXSCT * 8], mybir.dt.int16, tag="ix")
                nc.sync.dma_start(out=ixt[:, :sct * 8],
                                  in_=idx1[:, o0 * 8:(o0 + sct) * 8])
                dlt = mpool.tile([128, MAXSCT], BF, tag="dl")
                nc.sync.dma_start(out=dlt[:, :sct], in_=dl1[:, o0:o0 + sct])

                gbuf = gpool.tile([128, MAXSCT, CPAD], BF, tag="g")
                lq = 0
                for qi in range(NC):
                    nch = nch1[s][qi]
                    tbl = ag2_outA if qi < 2 else ag2_outB
                    nc.gpsimd.dma_gather(
                        gbuf[:, lq:lq + nch, :],
                        tbl[(qi % 2) * WS:(qi % 2 + 1) * WS, :],
                        ixt[:, lq * 8:(lq + nch) * 8],
                        nch * 128, nch * 128, CPAD,
                        single_packet=False, queue_num=next_q(),
                    )
                    lq += nch

                accs = [pspool.tile([128, CPAD], FP, space="PSUM",
                                    tag=f"acc{j}", name=f"acc{j}")
                        for j in range(SPB)]
                for qi in range(NC):
                    nch = nch1[s][qi]
                    lq = lbo1[s][qi][0]
                    ind = ipool.tile([128, MAXNQ, 128], BF, tag="i")
                    nc.vector.tensor_tensor(
                        out=ind[:, :nch, :],
                        in0=dlt[:, lq:lq + nch].to_broadcast([128, nch, 128]),
                        in1=iota_t[:, None, :].to_broadcast([128, nch, 128]),
                        op=mybir.AluOpType.is_equal,
                    )
                    for j in range(SPB):
                        bb = s * SPB + j
                        kq = int(kq1[bb, qi])
                        ps3 = accs[j]
                        boff = lbo1[s][qi][j]
                        for k in range(kq):
                            ck = boff + k
                            nc.tensor.matmul(
                                out=ps3[:, :],
                                lhsT=ind[:, ck - lq, :],
                                rhs=gbuf[:, ck, :],
                                start=(qi == 0 and k == 0),
                                stop=(qi == NC - 1 and k == kq - 1))

                og = opool.tile([128, SPB, CPAD], FP, tag="og")
                for j in range(SPB):
                    bb = s * SPB + j
                    ps3 = accs[j]
                    u = midpool.tile([128, CPAD], FP, tag="u")
                    nc.vector.tensor_tensor(
                        out=u[:, :], in0=ps3[:, :], in1=h2p_res[:, bb, :],
                        op=mybir.AluOpType.add)
                    v = midpool.tile([128, CPAD], FP, tag="v")
                    nc.scalar.activation(
                        out=v[:, :], in_=u[:, :],
                        func=mybir.ActivationFunctionType.Copy,
                        scale=dpcT_t[:, bb:bb + 1])
                    nc.vector.tensor_tensor(
                        out=og[:, j, :], in0=v[:, :], in1=b2_t[:, :],
                        op=mybir.AluOpType.add)
                nc.sync.dma_start(out=out_s[:, s * SPB:(s + 1) * SPB, :],
                                  in_=og[:, :, :])

    nc.compile()
    return nc


# ======================================================================
# kernel() entry point
# ======================================================================
import os as _os


LAST_EXEC_NS = None
LAST_RES = None


def kernel(x, edge_index, W1, b1, W2, b2):
    """Full-input GCN kernel: shards across 8 NeuronCores internally."""
    global LAST_EXEC_NS, LAST_RES
    import numpy as _np

    trace = bool(int(_os.environ.get("GCN_TRACE", "0")))
    if trace:
        try:
            import sys as _sys
            import types as _types
            from trn_agent_boot.trn_boot import _ntff_profile_via_ctypes
            if "antenv.axon_hooks" not in _sys.modules:
                _hook = _ntff_profile_via_ctypes("/opt/axon/libaxon_pjrt.so")
                _m = _types.ModuleType("antenv.axon_hooks")
                _m.get_axon_ntff_profile_hook = lambda: _hook
                _m.set_axon_ntff_profile_hook = lambda h: None
                _sys.modules["antenv.axon_hooks"] = _m
        except Exception:
            trace = False

    from concourse.bass_utils import run_bass_kernel_spmd

    cfg = Cfg()
    per_core, meta, _ = preprocess(cfg, x, edge_index, W1, b1, W2, b2)
    nc = build(cfg, meta)
    res = run_bass_kernel_spmd(
        nc, per_core, core_ids=list(range(cfg.W)), trace=trace,
    )
    LAST_EXEC_NS = res.exec_time_ns
    LAST_RES = res
    outs = [res.results[c]["out_s"] for c in range(cfg.W)]
    return _np.ascontiguousarray(postprocess(cfg, outs, meta).astype(_np.float32))


# revision 23
# speedup vs baseline: 1.3225x; 1.0706x over previous
"""GCN 2-layer kernel for trn2: host preprocessing + Bass kernel builder.

Math (per GCNConv, PyG-style):
  out = D^-1/2 (A+I) D^-1/2 (X W) + b
Layer1 -> relu -> Layer2.

Device plan (8 cores, SPMD), all tensors in pos-order with p-major DRAM
layout [128, NB, feat] (row of node (bb, sl) lives at [sl, bb, :]).
AllGathers are split into two half-collectives (blocks 0-48 / 49-97) so
each half overlaps compute; gather tables are the two half-outputs, each
addressed through 2 int16 windows (cores 0-3 / 4-7).

  P1: h1' = dinv .* (x_pos @ W1) -> resident h1p + ag1_inA/B stores
  AG1a (after group 6), AG1b (at end of P1)
  P3: per super (7 dst blocks): 4 dma_gather (one per (half, window)),
      is_equal indicator per (super, class), chunk matmuls accumulate in
      7 psum banks; self-loop via identity matmul from resident h1p;
      then per block: dinv scale -> relu+b1 -> @W2 -> dinv scale -> h2'
      (resident + stores to ag2_inA/B)
  AG2a (after super 6), AG2b (end of P3)
  P5: same gathers; chunk matmuls [dst, CPAD]; self-loop via resident
      h2' VE-add; dinv scale + b2 -> batched store to out_s
Host: unpermute rows, slice [:N0, :CLS].
"""

from dataclasses import dataclass

import numpy as np

import concourse.bass as bass
import concourse.mybir as mybir
import concourse.tile as tile
from concourse import bacc

FP = mybir.dt.float32
BF = mybir.dt.bfloat16


@dataclass
class Cfg:
    N0: int = 100000     # real nodes
    W: int = 8           # cores
    SHARD: int = 12544   # nodes per core (98 * 128)
    F: int = 512         # in features
    HID: int = 128
    CLS: int = 40
    CPAD: int = 128
    NC: int = 4          # gather classes (half, core-window)
    SPB: int = 7         # dst blocks per super
    GT: int = 7          # P1 col-tiles per load group

    @property
    def NP(self):
        return self.W * self.SHARD

    @property
    def NB(self):
        return self.SHARD // 128

    @property
    def BH(self):
        return self.NB // 2  # blocks per half (49)

    @property
    def RH(self):
        return 128 * self.BH  # rows per core per half table (6272)

    @property
    def WS(self):
        return 4 * self.RH   # rows per int16 window (25088)

    @property
    def NS(self):
        return self.NB // self.SPB


@dataclass
class Meta:
    kq1: np.ndarray = None   # [NB, NC] chunks per (block, class)
    kq2: np.ndarray = None
    node_of_pos: np.ndarray = None  # [W, SHARD] -> node id


def _route_edges(cfg, cls, lidx, dst_pos):
    """Bucket edges by (core of dst, super, class of src, block).
    Chunk stream order per core: (super, class, bb in super, chunk).
    Returns idx16 [W,128,CT*8], dl bf16 [W,128,CT], kq [NB,NC]."""
    import ml_dtypes
    W, SHARD, NC = cfg.W, cfg.SHARD, cfg.NC
    NB, SPB, NS = cfg.NB, cfg.SPB, cfg.NS

    c = dst_pos // SHARD
    r = dst_pos % SHARD
    bb = r // 128
    sl = r % 128

    key = (c * NB + bb) * NC + cls
    order = np.argsort(key, kind="stable")
    lidx_s = lidx[order]
    sl_s = sl[order]

    nseg = W * NB * NC
    counts = np.bincount(key[order], minlength=nseg).reshape(W, NB, NC)
    kq = np.maximum(
        np.ceil(counts.max(axis=0) / 128).astype(np.int64), 1)  # [NB, NC]

    # stream offset of (bb, class) segment: order (s, class, bb in s)
    seg_off = np.zeros((NB, NC), dtype=np.int64)
    cursor = 0
    for s in range(NS):
        for qi in range(NC):
            for j in range(SPB):
                b = s * SPB + j
                seg_off[b, qi] = cursor
                cursor += kq[b, qi] * 128
    CT = int(kq.sum())
    EPAD = CT * 128
    assert cursor == EPAD

    seg_start = np.zeros(nseg + 1, dtype=np.int64)
    seg_start[1:] = np.cumsum(counts.reshape(-1))

    idx16 = np.zeros((W, 128, CT * 8), dtype=np.int16)
    dl16 = np.zeros((W, 128, CT), dtype=ml_dtypes.bfloat16)
    for ci in range(W):
        idx_pad = np.zeros(EPAD, dtype=np.int64)
        dl_pad = np.full(EPAD, -1.0, dtype=np.float32)
        for bi in range(NB):
            for qi in range(NC):
                sidx = (ci * NB + bi) * NC + qi
                s0, s1 = seg_start[sidx], seg_start[sidx + 1]
                n = s1 - s0
                if n == 0:
                    continue
                o = seg_off[bi, qi]
                idx_pad[o:o + n] = lidx_s[s0:s1]
                dl_pad[o:o + n] = sl_s[s0:s1]
        a = idx_pad.reshape(CT, 8, 16)
        wrapped = a.transpose(2, 0, 1).reshape(16, CT * 8)
        idx16[ci] = np.tile(wrapped, (8, 1)).astype(np.int16)
        dl16[ci] = dl_pad.reshape(CT, 128).T.astype(ml_dtypes.bfloat16)
    return idx16, dl16, kq


def preprocess(cfg: Cfg, x, edge_index, W1, b1, W2, b2):
    import ml_dtypes
    N0, W, SHARD, NP = cfg.N0, cfg.W, cfg.SHARD, cfg.NP
    NB = cfg.NB
    x = np.asarray(x, dtype=np.float32)
    edge_index = np.asarray(edge_index)
    W1 = np.asarray(W1, np.float32)
    b1 = np.asarray(b1, np.float32)
    W2 = np.asarray(W2, np.float32)
    b2 = np.asarray(b2, np.float32)

    s = edge_index[0].astype(np.int64)
    d = edge_index[1].astype(np.int64)
    loops = np.arange(N0, dtype=np.int64)
    d_all = np.concatenate([d, loops])

    deg = np.bincount(d_all, minlength=NP).astype(np.float64)
    with np.errstate(divide="ignore"):
        dinv = np.where(deg > 0, 1.0 / np.sqrt(deg), 0.0).astype(np.float32)

    # degree-balanced serpentine block assignment over all W*NB blocks
    nblocks = W * NB
    order = np.argsort(-deg[:N0], kind="stable")
    all_ids = np.concatenate([order, np.arange(N0, NP, dtype=np.int64)])
    rr = np.arange(NP, dtype=np.int64)
    cyc = rr % (2 * nblocks)
    blk = np.where(cyc < nblocks, cyc, 2 * nblocks - 1 - cyc)
    slot_ctr = rr // (2 * nblocks) * 2 + (cyc >= nblocks).astype(np.int64)
    pos = (blk % W) * SHARD + (blk // W) * 128 + slot_ctr
    pos_of_node = np.empty(NP, dtype=np.int64)
    pos_of_node[all_ids] = pos
    node_of_pos = np.empty(NP, dtype=np.int64)
    node_of_pos[pos] = all_ids

    def gcls(p):
        """(class, lidx) of src pos: class = (bb//BH)*2 + c//4,
        lidx = (c%4)*RH + sl*BH + bb%BH."""
        BH, RH = cfg.BH, cfg.RH
        c = p // SHARD
        r = p % SHARD
        bb = r // 128
        sl = r % 128
        cls = (bb // BH) * 2 + c // 4
        lidx = (c % 4) * RH + sl * BH + (bb % BH)
        return cls, lidx

    # both layers exclude self-loops (handled on-device from residents)
    src_pos = pos_of_node[s]
    dst_pos = pos_of_node[d]
    cls, lidx = gcls(src_pos)
    idx16_1, dl1, kq1 = _route_edges(cfg, cls, lidx, dst_pos)
    idx16_2, dl2, kq2 = idx16_1, dl1, kq1

    dinv_pos = dinv[node_of_pos.reshape(W, SHARD)]  # [W, SHARD]

    xpad = np.zeros((NP, cfg.F), np.float32)
    xpad[:N0] = x
    ident = np.eye(128, dtype=np.float32)
    per_core = []
    for c in range(W):
        xs = xpad[node_of_pos[c * SHARD:(c + 1) * SHARD]]  # [SHARD, F] pos
        dpc = dinv_pos[c]
        inp = {
            "xT": np.ascontiguousarray(xs.T).astype(ml_dtypes.bfloat16),
            "w1": W1.astype(ml_dtypes.bfloat16),
            "b1col": b1.reshape(cfg.HID, 1).copy(),
            "w2p": np.pad(W2, ((0, 0), (0, cfg.CPAD - cfg.CLS))
                          ).astype(ml_dtypes.bfloat16),
            "b2rep": np.broadcast_to(
                np.pad(b2, (0, cfg.CPAD - cfg.CLS)), (128, cfg.CPAD)).copy(),
            "iota": np.broadcast_to(
                np.arange(128, dtype=np.float32),
                (128, 128)).astype(ml_dtypes.bfloat16),
            "ident": ident.astype(ml_dtypes.bfloat16),
            "idx1": idx16_1[c], "dl1": dl1[c],
            "dinv_pcT": np.ascontiguousarray(
                dpc.reshape(NB, 128).T).copy(),                 # [128, NB]
            "dinv_pr": np.broadcast_to(
                dpc, (128, SHARD)).astype(ml_dtypes.bfloat16),  # [128, SHARD]
        }
        per_core.append(inp)

    meta = Meta(kq1=kq1, kq2=kq2, node_of_pos=node_of_pos.reshape(W, SHARD))
    return per_core, meta, dinv


def postprocess(cfg: Cfg, outs, meta: Meta):
    """outs: list of [128, NB, CPAD] per core -> [N0, CLS] node order."""
    res = np.zeros((cfg.NP, cfg.CPAD), np.float32)
    for c in range(cfg.W):
        blockmaj = np.transpose(outs[c], (1, 0, 2)).reshape(
            cfg.SHARD, cfg.CPAD)
        res[meta.node_of_pos[c]] = blockmaj
    return res[:cfg.N0, :cfg.CLS]


def _superplan(cfg, kq):
    """Static per-super chunk layout: co[s], sct[s], nch[s][q], lbo[s][q][b]."""
    NS, SPB, NC = cfg.NS, cfg.SPB, cfg.NC
    co, sct, nch, lbo = [], [], [], []
    cursor = 0
    for s in range(NS):
        co.append(cursor)
        nq, lb = [], []
        loc = 0
        for qi in range(NC):
            lbq = []
            for j in range(SPB):
                lbq.append(loc)
                loc += int(kq[s * SPB + j, qi])
            lb.append(lbq)
            nq.append(sum(int(kq[s * SPB + j, qi]) for j in range(SPB)))
        nch.append(nq)
        lbo.append(lb)
        sct.append(loc)
        cursor += loc
    return co, sct, nch, lbo


def build(cfg: Cfg, meta: Meta):
    W, SHARD, NP, F = cfg.W, cfg.SHARD, cfg.NP, cfg.F
    HID, CPAD, NB, BH, RH, WS = (cfg.HID, cfg.CPAD, cfg.NB, cfg.BH, cfg.RH,
                                 cfg.WS)
    NC, SPB, NS, GT = cfg.NC, cfg.SPB, cfg.NS, cfg.GT
    kq1 = meta.kq1
    CT1 = int(kq1.sum())
    KT = F // 128
    GC = GT * 128  # cols per P1 load group
    NG = SHARD // GC
    NH = W * RH    # rows per half table
    HGRP = NS // 2  # supers per half (7)
    NH = W * RH    # rows per half table
    HGRP = NS // 2  # supers per half (7)

    co1, sct1, nch1, lbo1 = _superplan(cfg, kq1)
    MAXSCT = max(sct1)
    MAXNQ = max(max(r) for r in nch1)

    nc = bacc.Bacc("TRN2", target_bir_lowering=False, debug=False,
                   num_devices=W, num_swdge_queues=4)

    xT = nc.dram_tensor("xT", [F, SHARD], BF, kind="ExternalInput")
    w1 = nc.dram_tensor("w1", [F, HID], BF, kind="ExternalInput")
    b1col = nc.dram_tensor("b1col", [HID, 1], FP, kind="ExternalInput")
    w2p = nc.dram_tensor("w2p", [HID, CPAD], BF, kind="ExternalInput")
    b2rep = nc.dram_tensor("b2rep", [128, CPAD], FP, kind="ExternalInput")
    iota = nc.dram_tensor("iota", [128, 128], BF, kind="ExternalInput")
    identt = nc.dram_tensor("ident", [128, 128], BF, kind="ExternalInput")
    idx1 = nc.dram_tensor("idx1", [128, CT1 * 8], mybir.dt.int16,
                          kind="ExternalInput")
    dl1 = nc.dram_tensor("dl1", [128, CT1], BF, kind="ExternalInput")
    dinv_pcT = nc.dram_tensor("dinv_pcT", [128, NB], FP, kind="ExternalInput")
    dinv_pr = nc.dram_tensor("dinv_pr", [128, SHARD], BF, kind="ExternalInput")
    out_s = nc.dram_tensor("out_s", [128, NB, CPAD], FP, kind="ExternalOutput")

    ag1_inA = nc.dram_tensor("ag1_inA", [128, BH, HID], BF)
    ag1_inB = nc.dram_tensor("ag1_inB", [128, BH, HID], BF)
    ag1_outA = nc.dram_tensor("ag1_outA", [NH, HID], BF, addr_space="Shared")
    ag1_outB = nc.dram_tensor("ag1_outB", [NH, HID], BF, addr_space="Shared")
    ag2_inA = nc.dram_tensor("ag2_inA", [128, BH, CPAD], BF)
    ag2_inB = nc.dram_tensor("ag2_inB", [128, BH, CPAD], BF)
    ag2_outA = nc.dram_tensor("ag2_outA", [NH, CPAD], BF, addr_space="Shared")
    ag2_outB = nc.dram_tensor("ag2_outB", [NH, CPAD], BF, addr_space="Shared")

    def ag(ins_ap, outs_ap):
        nc.gpsimd.collective_compute(
            "AllGather", mybir.AluOpType.bypass,
            replica_groups=[list(range(W))],
            ins=[ins_ap], outs=[outs_ap],
        )

    with tile.TileContext(nc) as tc:
        with (
            tc.tile_pool(name="const", bufs=1) as cpool,
            tc.tile_pool(name="p1x", bufs=2) as p1pool,
            tc.tile_pool(name="meta1", bufs=3) as mpool,
            tc.tile_pool(name="gath", bufs=3) as gpool,
            tc.tile_pool(name="indp", bufs=2) as ipool,
            tc.tile_pool(name="mid", bufs=2) as midpool,
            tc.tile_pool(name="dprp", bufs=2) as dpool,
            tc.tile_pool(name="outg", bufs=1) as opool,
            tc.tile_pool(name="ps", bufs=1, space="PSUM") as pspool,
        ):
            # ---- constants ----
            iota_t = cpool.tile([128, 128], BF)
            nc.sync.dma_start(out=iota_t[:, :], in_=iota[:, :])
            ident_t = cpool.tile([128, 128], BF)
            nc.sync.dma_start(out=ident_t[:, :], in_=identt[:, :])
            b1_t = cpool.tile([HID, 1], FP)
            nc.sync.dma_start(out=b1_t[:, :], in_=b1col[:, :])
            w2_t = cpool.tile([HID, CPAD], BF)
            nc.sync.dma_start(out=w2_t[:, :], in_=w2p[:, :])
            b2_t = cpool.tile([128, CPAD], FP)
            nc.sync.dma_start(out=b2_t[:, :], in_=b2rep[:, :])
            w1k_t = cpool.tile([128, KT, HID], BF)
            for k in range(KT):
                nc.sync.dma_start(out=w1k_t[:, k, :],
                                  in_=w1[k * 128:(k + 1) * 128, :])
            dpcT_t = cpool.tile([128, NB], FP)
            nc.sync.dma_start(out=dpcT_t[:, :], in_=dinv_pcT[:, :])
            h1p_res = cpool.tile([128, NB, HID], BF)
            h2p_res = cpool.tile([128, NB, CPAD], BF)

            qctr = [0]

            def next_q():
                qctr[0] = (qctr[0] + 1) % 4
                return qctr[0]

            # ---- P1: h1' = dinv .* (x @ W1), streamed col groups ----
            for g in range(NG):
                xt = p1pool.tile([128, KT, GC], BF, tag="xt")
                for k in range(KT):
                    nc.sync.dma_start(
                        out=xt[:, k, :],
                        in_=xT[k * 128:(k + 1) * 128, g * GC:(g + 1) * GC])
                for t in range(GT):
                    blk = g * GT + t
                    psh = pspool.tile([128, HID], FP, space="PSUM",
                                      tag=f"acc{t}", name=f"acc{t}")
                    for k in range(KT):
                        nc.tensor.matmul(
                            out=psh[:, :],
                            lhsT=xt[:, k, t * 128:(t + 1) * 128],
                            rhs=w1k_t[:, k, :],
                            start=(k == 0), stop=(k == KT - 1))
                    nc.scalar.activation(
                        out=h1p_res[:, blk, :], in_=psh[:, :],
                        func=mybir.ActivationFunctionType.Copy,
                        scale=dpcT_t[:, blk:blk + 1])
                if g < HGRP:
                    nc.scalar.dma_start(
                        out=ag1_inA[:, g * GT:(g + 1) * GT, :],
                        in_=h1p_res[:, g * GT:(g + 1) * GT, :])
                else:
                    nc.scalar.dma_start(
                        out=ag1_inB[:, (g - HGRP) * GT:(g - HGRP + 1) * GT, :],
                        in_=h1p_res[:, g * GT:(g + 1) * GT, :])
                if g == HGRP - 1:
                    ag(ag1_inA[:, :, :], ag1_outA[:, :])
            ag(ag1_inB[:, :, :], ag1_outB[:, :])

            # ---- P3: L1 aggregation + relu + @W2 -> h2' ----
            for s in range(NS):
                sct = sct1[s]
                o0 = co1[s]
                ixt = mpool.tile([128, MAXSCT * 8], mybir.dt.int16, tag="ix")
                nc.sync.dma_start(out=ixt[:, :sct * 8],
                                  in_=idx1[:, o0 * 8:(o0 + sct) * 8])
                dlt = mpool.tile([128, MAXSCT], BF, tag="dl")
                nc.sync.dma_start(out=dlt[:, :sct], in_=dl1[:, o0:o0 + sct])
                dprs = dpool.tile([128, SPB * 128], BF, tag="dprs")
                nc.sync.dma_start(
                    out=dprs[:, :],
                    in_=dinv_pr[:, s * SPB * 128:(s + 1) * SPB * 128])

                gbuf = gpool.tile([128, MAXSCT, HID], BF, tag="g")
                lq = 0
                for qi in range(NC):
                    nch = nch1[s][qi]
                    tbl = ag1_outA if qi < 2 else ag1_outB
                    nc.gpsimd.dma_gather(
                        gbuf[:, lq:lq + nch, :],
                        tbl[(qi % 2) * WS:(qi % 2 + 1) * WS, :],
                        ixt[:, lq * 8:(lq + nch) * 8],
                        nch * 128, nch * 128, HID,
                        single_packet=False, queue_num=next_q(),
                    )
                    lq += nch

                accs = [pspool.tile([128, 128], FP, space="PSUM",
                                    tag=f"acc{j}", name=f"acc{j}")
                        for j in range(SPB)]
                # self-loop: ps1 := h1p_blk^T via identity matmul
                for j in range(SPB):
                    bb = s * SPB + j
                    nc.tensor.matmul(out=accs[j][:, :],
                                     lhsT=h1p_res[:, bb, :],
                                     rhs=ident_t[:, :],
                                     start=True, stop=False)
                for qi in range(NC):
                    nch = nch1[s][qi]
                    lq = lbo1[s][qi][0]
                    ind = ipool.tile([128, MAXNQ, 128], BF, tag="i")
                    nc.vector.tensor_tensor(
                        out=ind[:, :nch, :],
                        in0=dlt[:, lq:lq + nch].to_broadcast([128, nch, 128]),
                        in1=iota_t[:, None, :].to_broadcast([128, nch, 128]),
                        op=mybir.AluOpType.is_equal,
                    )
                    for j in range(SPB):
                        bb = s * SPB + j
                        kq = int(kq1[bb, qi])
                        ps1 = accs[j]
                        boff = lbo1[s][qi][j]
                        for k in range(kq):
                            ck = boff + k
                            nc.tensor.matmul(
                                out=ps1[:, :],
                                lhsT=gbuf[:, ck, :],
                                rhs=ind[:, ck - lq, :],
                                start=False,
                                stop=(qi == NC - 1 and k == kq - 1))

                for j in range(SPB):
                    bb = s * SPB + j
                    ps1 = accs[j]
                    t1 = midpool.tile([128, 128], FP, tag="t1")
                    nc.vector.tensor_tensor(
                        out=t1[:, :], in0=ps1[:, :],
                        in1=dprs[:, j * 128:(j + 1) * 128],
                        op=mybir.AluOpType.mult)
                    r1 = midpool.tile([128, 128], BF, tag="r1")
                    nc.scalar.activation(
                        out=r1[:, :], in_=t1[:, :],
                        func=mybir.ActivationFunctionType.Relu,
                        bias=b1_t[:, :1])
                    ps2 = pspool.tile([128, CPAD], FP, space="PSUM",
                                       tag=f"acc{j}", name=f"accb{j}")
                    nc.tensor.matmul(out=ps2[:, :], lhsT=r1[:, :],
                                     rhs=w2_t[:, :], start=True, stop=True)
                    nc.scalar.activation(
                        out=h2p_res[:, bb, :], in_=ps2[:, :],
                        func=mybir.ActivationFunctionType.Copy,
                        scale=dpcT_t[:, bb:bb + 1])
                if s < HGRP:
                    nc.scalar.dma_start(
                        out=ag2_inA[:, s * SPB:(s + 1) * SPB, :],
                        in_=h2p_res[:, s * SPB:(s + 1) * SPB, :])
                else:
                    sb = s - HGRP
                    nc.scalar.dma_start(
                        out=ag2_inB[:, sb * SPB:(sb + 1) * SPB, :],
                        in_=h2p_res[:, s * SPB:(s + 1) * SPB, :])
                if s == HGRP - 1:
                    ag(ag2_inA[:, :, :], ag2_outA[:, :])
            ag(ag2_inB[:, :, :], ag2_outB[:, :])

            # ---- P5: L2 aggregation + self-loop + b2 -> out ----
            for s in range(NS):
                sct = sct1[s]
                o0 = co1[s]
                ixt = mpool.tile([128, MAXSCT * 8], mybir.dt.int16, tag="ix")
                nc.sync.dma_start(out=ixt[:, :sct * 8],
                                  in_=idx1[:, o0 * 8:(o0 + sct) * 8])
                dlt = mpool.tile([128, MAXSCT], BF, tag="dl")
                nc.sync.dma_start(out=dlt[:, :sct], in_=dl1[:, o0:o0 + sct])

                gbuf = gpool.tile([128, MAXSCT, CPAD], BF, tag="g")
                lq = 0
                for qi in range(NC):
                    nch = nch1[s][qi]
                    tbl = ag2_outA if qi < 2 else ag2_outB
                    nc.gpsimd.dma_gather(
                        gbuf[:, lq:lq + nch, :],
                        tbl[(qi % 2) * WS:(qi % 2 + 1) * WS, :],
                        ixt[:, lq * 8:(lq + nch) * 8],
                        nch * 128, nch * 128, CPAD,
                        single_packet=False, queue_num=next_q(),
                    )
                    lq += nch

                accs = [pspool.tile([128, CPAD], FP, space="PSUM",
                                    tag=f"acc{j}", name=f"acc{j}")
                        for j in range(SPB)]
                for qi in range(NC):
                    nch = nch1[s][qi]
                    lq = lbo1[s][qi][0]
                    ind = ipool.tile([128, MAXNQ, 128], BF, tag="i")
                    nc.vector.tensor_tensor(
                        out=ind[:, :nch, :],
                        in0=dlt[:, lq:lq + nch].to_broadcast([128, nch, 128]),
                        in1=iota_t[:, None, :].to_broadcast([128, nch, 128]),
                        op=mybir.AluOpType.is_equal,
                    )
                    for j in range(SPB):
                        bb = s * SPB + j
                        kq = int(kq1[bb, qi])
                        ps3 = accs[j]
                        boff = lbo1[s][qi][j]
                        for k in range(kq):
                            ck = boff + k
                            nc.tensor.matmul(
                                out=ps3[:, :],
                                lhsT=ind[:, ck - lq, :],
                                rhs=gbuf[:, ck, :],
                                start=(qi == 0 and k == 0),
                                stop=(qi == NC - 1 and k == kq - 1))

                og = opool.tile([128, SPB, CPAD], FP, tag="og")
                for j in range(SPB):
                    bb = s * SPB + j
                    ps3 = accs[j]
                    u = midpool.tile([128, CPAD], FP, tag="u")
                    nc.vector.tensor_tensor(
                        out=u[:, :], in0=ps3[:, :], in1=h2p_res[:, bb, :],
                        op=mybir.AluOpType.add)
                    v = midpool.tile([128, CPAD], FP, tag="v")
                    nc.scalar.activation(
                        out=v[:, :], in_=u[:, :],
                        func=mybir.ActivationFunctionType.Copy,
                        scale=dpcT_t[:, bb:bb + 1])
                    nc.vector.tensor_tensor(
                        out=og[:, j, :], in0=v[:, :], in1=b2_t[:, :],
                        op=mybir.AluOpType.add)
                nc.scalar.dma_start(out=out_s[:, s * SPB:(s + 1) * SPB, :],
                                    in_=og[:, :, :])

    nc.compile()
    return nc


# ======================================================================
# kernel() entry point
# ======================================================================
import os as _os


LAST_EXEC_NS = None
LAST_RES = None


def kernel(x, edge_index, W1, b1, W2, b2):
    """Full-input GCN kernel: shards across 8 NeuronCores internally."""
    global LAST_EXEC_NS, LAST_RES
    import numpy as _np

    trace = bool(int(_os.environ.get("GCN_TRACE", "0")))
    if trace:
        try:
            import sys as _sys
            import types as _types
            from trn_agent_boot.trn_boot import _ntff_profile_via_ctypes
            if "antenv.axon_hooks" not in _sys.modules:
                _hook = _ntff_profile_via_ctypes("/opt/axon/libaxon_pjrt.so")
                _m = _types.ModuleType("antenv.axon_hooks")
                _m.get_axon_ntff_profile_hook = lambda: _hook
                _m.set_axon_ntff_profile_hook = lambda h: None
                _sys.modules["antenv.axon_hooks"] = _m
        except Exception:
            trace = False

    from concourse.bass_utils import run_bass_kernel_spmd

    cfg = Cfg()
    per_core, meta, _ = preprocess(cfg, x, edge_index, W1, b1, W2, b2)
    nc = build(cfg, meta)
    res = run_bass_kernel_spmd(
        nc, per_core, core_ids=list(range(cfg.W)), trace=trace,
    )
    LAST_EXEC_NS = res.exec_time_ns
    LAST_RES = res
    outs = [res.results[c]["out_s"] for c in range(cfg.W)]
    return _np.ascontiguousarray(postprocess(cfg, outs, meta).astype(_np.float32))
